# revision 21
# baseline (speedup 1.0000x reference)
"""Trainium2 Bass kernel for multi-head attention (B=2, T=2048, D=1024, H=16).

Sharding (Megatron-style): 8 cores = 2 batches x 4 head-groups. Core c handles
batch c//4 and heads [4*(c%4), 4*(c%4)+4): WQ/WK/WV split column-wise (256
cols per core), WO split row-wise. Each core writes a (T, D) fp16 partial
output; the host sums the 4 partials per batch.

Device schedule (fp16 matmul operands, fp32 PSUM):
- A1/A2: Q^T/K^T projections in [d, q] layout (d on partitions), m-outer so
  the PE chases the X DMAs; PSUM->SBUF copies on DVE so ACT stays exp-only.
- Attention is one global conveyor over units (i, hp, g, hl):
  ST (PE, two heads packed into 64-row PE groups via tile_position) ->
  exp (ACT) -> causal mask (gpsimd affine_select, diagonal units only) ->
  PV (PE; V carries a ones column so PSUM row DK accumulates the softmax
  denominator). PV lags ST by DEPTH units, and V-projection/out-projection
  matmuls are interleaved as PE filler, so the ACT-bound softmax never
  leaves the PE idle (keeps the HAM clock warm).
- Per-head normalization starts the moment that head's PV finishes:
  denominator row -> reciprocal_approx_fast (DVE) -> partition_broadcast
  (gpsimd) -> multiply (DVE) -> out-proj granules enqueued as filler.
"""

from collections import deque

import numpy as np

import concourse.mybir as mybir
import concourse.tile as tile
from concourse import bacc
from concourse.bass_utils import run_bass_kernel_spmd
from concourse._compat import get_trn_type

F32 = mybir.dt.float32
F32R = mybir.dt.float32r
F16 = mybir.dt.float16
AF = mybir.ActivationFunctionType
ALU = mybir.AluOpType

B, T, D, H = 2, 2048, 1024, 16
DK = 64
NCORES = 8
GROUPS = 4          # head-groups = cores per batch
DG = 256            # d-columns per core (4 heads x 64)
NH = 4              # heads per core
QB = 512            # query block
NQB = T // QB       # 4
KTILE = 128
NKT = T // KTILE    # 16
NMT = D // 128      # 8 contraction tiles over D
GK = 2              # k-tiles per conveyor unit
DEPTH = 4           # PV lag in conveyor units


def build_program():
    nc = bacc.Bacc(get_trn_type() or "TRN2", target_bir_lowering=False, debug=False)

    x1t = nc.dram_tensor("x1t", [D, T], F16, kind="ExternalInput").ap()
    x2t = nc.dram_tensor("x2t", [D, T], F16, kind="ExternalInput").ap()
    wq = nc.dram_tensor("wq", [128, NMT, DG], F16, kind="ExternalInput").ap()
    wk = nc.dram_tensor("wk", [128, NMT, DG], F16, kind="ExternalInput").ap()
    wv = nc.dram_tensor("wv", [128, NMT, DG], F16, kind="ExternalInput").ap()
    wo = nc.dram_tensor("wo", [128, 2, D], F16, kind="ExternalInput").ap()
    out = nc.dram_tensor("out", [T, D], F16, kind="ExternalOutput").ap()

    with tile.TileContext(nc) as tc:
        _emit(nc, tc, x1t, x2t, wq, wk, wv, wo, out)
    nc.compile()
    return nc


def _emit(nc, tc, x1t, x2t, wq, wk, wv, wo, out):
    from contextlib import ExitStack

    with ExitStack() as ctx:
        wpool = ctx.enter_context(tc.tile_pool(name="weights", bufs=1))
        qkv = ctx.enter_context(tc.tile_pool(name="qkv", bufs=1))

        # --- weights to SBUF (gpsimd queue; wq first: it gates the first MMs)
        wq_sb = wpool.tile([128, NMT, DG], F16)
        nc.gpsimd.dma_start(wq_sb[:], wq[:])
        wk_sb = wpool.tile([128, NMT, DG], F16)
        nc.gpsimd.dma_start(wk_sb[:], wk[:])
        wv_sb = wpool.tile([128, NMT, DG], F16)
        nc.gpsimd.dma_start(wv_sb[:], wv[:])
        wo_sb = wpool.tile([128, 2, D], F16)
        nc.gpsimd.dma_start(wo_sb[:], wo[:])

        # Residents: QT/KT as [128, dt, q]; V as per-j tiles [k, h, DK+1]
        qt_sb = qkv.tile([128, 2, T], F16)
        kt_sb = qkv.tile([128, 2, T], F16)
        v_sb = [qkv.tile([128, NH, DK + 1], F16, name=f"vsb{j}") for j in range(NKT)]
        for j in range(NKT):
            nc.gpsimd.memset(v_sb[j][:, :, DK : DK + 1], 1.0)

        # X2 resident (V-projection granules run inside the conveyor).
        # x2 DMAs go on the sync queue AFTER x1's so the two streams don't
        # halve each other's HBM bandwidth (A1 is gated by x1 alone).
        x2_sb = qkv.tile([128, NMT, T], F16)

        # ones row for the PE-side denominator broadcast (K=1 f16 matmul)
        ones_sb = wpool.tile([1, DK], F16)
        nc.vector.memset(ones_sb[:], 1.0)

        # --- A1: QT = WQ^T X1^T, m-outer so matmuls chase the x1 DMAs ---
        with tc.tile_pool(name="x1pool", bufs=1) as x1pool, tc.tile_pool(
            name="psA", bufs=1, space="PSUM"
        ) as psA:
            qps = [psA.tile([128, NQB, QB], F32, name=f"qps{t}") for t in range(2)]
            x1_sb = x1pool.tile([128, NMT, T], F16)
            for m in range(NMT):
                nc.sync.dma_start(x1_sb[:, m, :], x1t[m * 128 : (m + 1) * 128, :])
            for m in range(NMT):  # x2 queued behind all of x1
                nc.sync.dma_start(x2_sb[:, m, :], x2t[m * 128 : (m + 1) * 128, :])
            for m in range(NMT):
                for dt in range(2):
                    lhsT = wq_sb[:, m, dt * 128 : (dt + 1) * 128]
                    for qc in range(NQB):
                        nc.tensor.matmul(
                            qps[dt][:, qc, :],
                            lhsT,
                            x1_sb[:, m, qc * QB : (qc + 1) * QB],
                            start=(m == 0),
                            stop=(m == NMT - 1),
                        )
            for qc in range(NQB):
                for dt in range(2):
                    nc.vector.tensor_copy(
                        out=qt_sb[:, dt, qc * QB : (qc + 1) * QB],
                        in_=qps[dt][:, qc, :],
                    )

        # --- A2: KT (x2 resident by now) ---
        with tc.tile_pool(name="psK", bufs=1, space="PSUM") as psK:
            kps = [psK.tile([128, NQB, QB], F32, name=f"kps{t}") for t in range(2)]
            for m in range(NMT):
                for dt in range(2):
                    lhsT = wk_sb[:, m, dt * 128 : (dt + 1) * 128]
                    for kc in range(NQB):
                        nc.tensor.matmul(
                            kps[dt][:, kc, :],
                            lhsT,
                            x2_sb[:, m, kc * QB : (kc + 1) * QB],
                            start=(m == 0),
                            stop=(m == NMT - 1),
                        )
            for kc in range(NQB):
                for dt in range(2):
                    nc.vector.tensor_copy(
                        out=kt_sb[:, dt, kc * QB : (kc + 1) * QB],
                        in_=kps[dt][:, kc, :],
                    )

        # --- attention-era pools (PSUM: 4 + 2 + 1 + 1 = 8 banks) ---
        st_ps = ctx.enter_context(tc.tile_pool(name="st_ps", bufs=2, space="PSUM"))
        ct_ps = ctx.enter_context(tc.tile_pool(name="ct_ps", bufs=1, space="PSUM"))
        op_ps = ctx.enter_context(tc.tile_pool(name="op_ps", bufs=1, space="PSUM"))
        vp_ps = ctx.enter_context(tc.tile_pool(name="vp_ps", bufs=1, space="PSUM"))
        pt_pool = ctx.enter_context(tc.tile_pool(name="pt", bufs=6))
        lin_pool = ctx.enter_context(tc.tile_pool(name="lin", bufs=4))
        ctu_pool = ctx.enter_context(tc.tile_pool(name="ctu", bufs=4))
        ctn_pool = ctx.enter_context(tc.tile_pool(name="ctn", bufs=2))
        osb_pool = ctx.enter_context(tc.tile_pool(name="osb", bufs=2))

        # ---- V-projection granules (two 4-MM halves per k-tile j) ----
        vp_state = {}

        def vproj_half(j, half):
            if half == 0:
                vp_state[j] = vp_ps.tile([128, QB], F32, name="vps")
            vps = vp_state[j]
            for m in range(half * 4, half * 4 + 4):
                nc.tensor.matmul(
                    vps[:, 0:DG],
                    x2_sb[:, m, j * 128 : (j + 1) * 128],
                    wv_sb[:, m, :],
                    start=(m == 0),
                    stop=(m == NMT - 1),
                )
            if half == 1:
                for h in range(NH):
                    nc.vector.tensor_copy(
                        out=v_sb[j][:, h, 0:DK], in_=vps[:, h * DK : (h + 1) * DK]
                    )
                del vp_state[j]

        for j in range(4):  # needed by block i=0, before the conveyor
            vproj_half(j, 0)
            vproj_half(j, 1)

        # ---- out-projection granules (2 accumulating MMs + copy + DMA);
        # granules alternate between the op and vproj PSUM banks so
        # consecutive granules double-buffer without an extra bank ----
        op_count = [0]

        def outproj_granule(i, ctn_sb, qs, nch):
            op_count[0] += 1
            if op_count[0] % 2 == 0:
                ops = op_ps.tile([128, QB], F32, name="ops")
            else:
                ops = vp_ps.tile([128, QB], F32, name="vps")
            for dt in range(2):
                nc.tensor.matmul(
                    ops[:],
                    ctn_sb[:, dt, qs * 128 : (qs + 1) * 128],
                    wo_sb[:, dt, nch * QB : (nch + 1) * QB],
                    start=(dt == 0),
                    stop=(dt == 1),
                )
            osb = osb_pool.tile([128, QB], F16, name="osb")
            nc.vector.tensor_copy(out=osb[:], in_=ops[:])
            nc.sync.dma_start(
                out[
                    i * QB + qs * 128 : i * QB + (qs + 1) * 128,
                    nch * QB : (nch + 1) * QB,
                ],
                osb[:],
            )

        # ---- conveyor state ----
        fillers = deque()
        pend = deque()
        normq = deque()  # (enqueue_unit_idx, closure) - run >=2 units later
        ctp_cur = {}   # hp -> [ctp_hl0, ctp_hl1] for the active (i, hp)
        ctn_cur = {}   # i -> ctn tile

        def drain_norm(i, hp, hl, ctp, uidx):
            """Head (2*hp+hl) of block i: reciprocal of the denominator row,
            then (deferred) PE-broadcast into rows 64:128 of the same PSUM
            bank and a fused PSUM-read multiply into ctn."""
            if i not in ctn_cur:
                ctn_cur[i] = ctn_pool.tile([128, 2, QB], F16, name="ctn")
            ctn_sb = ctn_cur[i]
            lrow = lin_pool.tile([1, QB], F32, name="lrow")
            nc.vector.tensor_copy(out=lrow[:], in_=ctp[DK : DK + 1, :])
            linv = lin_pool.tile([1, QB], F32, name="linv")
            # NB: approx_fast misreads PSUM operands on HW - keep in_ in SBUF
            nc.vector.reciprocal_approx_fast(out=linv[:], in_=lrow[:])
            linv16 = lin_pool.tile([1, QB], F16, name="linv16")
            nc.vector.tensor_copy(out=linv16[:], in_=linv[:])
            ctu = ctu_pool.tile([DK, QB], F32, name="ctu")
            nc.vector.tensor_copy(out=ctu[:], in_=ctp[0:DK, :])

            def norm():
                # broadcast 1/denom across partitions via K=1 f16 matmul
                # into the upper half of the ctp bank (one PSUM operand max
                # per DVE op, so ctx was copied to SBUF above)
                nc.tensor.matmul(
                    ctp[DK : DK + DK, :],
                    ones_sb[:],
                    linv16[:],
                    start=True,
                    stop=True,
                    tile_position=(0, 64),
                )
                nc.vector.tensor_tensor(
                    ctn_sb[hl * DK : (hl + 1) * DK, hp, :],
                    ctu[:],
                    ctp[DK : DK + DK, :],
                    ALU.mult,
                )
                if hp == 1 and hl == 1:  # block i fully normalized -> out-proj
                    for qs in range(4):
                        for nch in range(2):
                            fillers.append(
                                lambda i=i, c=ctn_sb, qs=qs, nch=nch: outproj_granule(
                                    i, c, qs, nch
                                )
                            )

            normq.append((uidx, norm))

        def emit_pv(u, pt, uidx):
            i, hp, g, hl = u
            njt = 4 * (i + 1)
            if g == 0 and hl == 0:
                ctp_cur[hp] = [
                    ct_ps.tile([128, QB], F32, name=f"ctp{t}") for t in range(2)
                ]
            ctp = ctp_cur[hp][hl]
            for jj in range(GK):
                j = g * GK + jj
                nc.tensor.matmul(
                    ctp[0 : DK + 1, :],
                    v_sb[j][:, hp * 2 + hl, :],
                    pt[:, jj, :],
                    start=(j == 0),
                    stop=(j == njt - 1),
                )
            if g == 2 * i + 1:  # this head's last PV -> normalize
                drain_norm(i, hp, hl, ctp, uidx)

        gunits = []
        for i in range(NQB):
            for hp in range(2):
                for g in range(2 * i + 2):
                    gunits.append((i, hp, g))

        cur_block = -1
        uidx = 0
        for i, hp, g in gunits:
            if i != cur_block:
                cur_block = i
                # V-projection for the NEXT block's k-tiles rides as filler
                for j in range(4 * (i + 1), min(4 * (i + 2), NKT)):
                    fillers.append(lambda j=j: vproj_half(j, 0))
                    fillers.append(lambda j=j: vproj_half(j, 1))
            # Both heads' STs adjacent in the PE stream: the two 64-row
            # PE groups then execute concurrently (row-group pairing),
            # and the ACT stream is fed two exps per g-unit.
            for hl in range(2):
                lo, hi = hl * 64, hl * 64 + 64
                stm = st_ps.tile([128, GK, QB], F32, name="stm")
                for jj in range(GK):
                    j = g * GK + jj
                    nc.tensor.matmul(
                        stm[:, jj, :],
                        kt_sb[lo:hi, hp, j * 128 : (j + 1) * 128],
                        qt_sb[lo:hi, hp, i * QB : (i + 1) * QB],
                        start=True,
                        stop=True,
                        tile_position=(lo, 0),
                    )
                pt = pt_pool.tile([128, GK, QB], F16)
                trim = max(0, 128 * (g * GK - 4 * i))  # no q < trim unmasked
                if trim > 0:
                    # exp only the potentially-valid q range; affine_select
                    # below zero-fills wherever its predicate is false,
                    # which covers the untouched q < trim region.
                    nc.scalar.activation(
                        pt[:, :, trim:QB], stm[:, :, trim:QB], AF.Exp
                    )
                else:
                    nc.scalar.activation(pt[:], stm[:], AF.Exp)
                if g * GK >= 4 * i:
                    # diagonal group: keep (p, jj, f) iff
                    # f - p - 128*(g*GK - 4i) - 128*jj >= 0
                    nc.gpsimd.affine_select(
                        pt[:],
                        pt[:],
                        pattern=[[-128, GK], [1, QB]],
                        compare_op=ALU.is_ge,
                        fill=0.0,
                        base=-(128 * (g * GK - 4 * i)),
                        channel_multiplier=-1,
                    )
                pend.append(((i, hp, g, hl), pt))
            uidx += 2
            while len(pend) > DEPTH:
                emit_pv(*pend.popleft(), uidx)
            if normq and normq[0][0] <= uidx - 2:
                normq.popleft()[1]()
            for _ in range(2):
                if fillers:
                    fillers.popleft()()

        while pend:
            emit_pv(*pend.popleft(), uidx)
            uidx += 1
        while normq:
            normq.popleft()[1]()
        while fillers:  # out-proj of the last block
            fillers.popleft()()


_PROG = None


def _get_prog():
    global _PROG
    if _PROG is None:
        _PROG = build_program()
    return _PROG


def _wlayout(w):
    """[(n_out*128), f] -> [128, n_out, f] contiguous fp16 (device lhsT layout)."""
    n_out = w.shape[0] // 128
    return np.ascontiguousarray(
        w.reshape(n_out, 128, w.shape[1]).transpose(1, 0, 2)
    ).astype(np.float16)


def shard_inputs(X1, X2, WQ, WK, WV, WO):
    X1 = np.asarray(X1, dtype=np.float32)
    X2 = np.asarray(X2, dtype=np.float32)
    WQ = np.asarray(WQ, dtype=np.float32)
    WK = np.asarray(WK, dtype=np.float32)
    WV = np.asarray(WV, dtype=np.float32)
    WO = np.asarray(WO, dtype=np.float32)
    x1t = [np.ascontiguousarray(X1[b].T).astype(np.float16) for b in range(B)]
    x2t = [np.ascontiguousarray(X2[b].T).astype(np.float16) for b in range(B)]
    in_maps = []
    for c in range(NCORES):
        b, g = divmod(c, GROUPS)
        sl = slice(g * DG, (g + 1) * DG)
        in_maps.append(
            {
                "x1t": x1t[b],
                "x2t": x2t[b],
                # fold the 1/sqrt(DK) score scale into WQ (exact: power of 2)
                "wq": _wlayout(WQ[:, sl] * np.float32(0.125)),
                "wk": _wlayout(WK[:, sl]),
                "wv": _wlayout(WV[:, sl]),
                "wo": _wlayout(WO[sl, :]),
            }
        )
    return in_maps


LAST_RESULT = None


def kernel(X1, X2, padding_mask, WQ, WK, WV, WO, **kwargs):
    global LAST_RESULT
    del padding_mask  # all-False by construction (spec fill: zeros)
    nc = _get_prog()
    in_maps = shard_inputs(X1, X2, WQ, WK, WV, WO)
    res = run_bass_kernel_spmd(nc, in_maps, list(range(NCORES)), **kwargs)
    LAST_RESULT = res
    out = np.zeros((B, T, D), dtype=np.float32)
    for c in range(NCORES):
        out[c // GROUPS] += res.results[c]["out"]
    return out


# revision 22
# speedup vs baseline: 1.0863x; 1.0863x over previous
"""Trainium2 Bass kernel for multi-head attention (B=2, T=2048, D=1024, H=16).

Sharding (Megatron-style): 8 cores = 2 batches x 4 head-groups. Core c handles
batch c//4 and heads [4*(c%4), 4*(c%4)+4): WQ/WK/WV split column-wise (256
cols per core), WO split row-wise. Each core writes a (T, D) fp16 partial
output; the host sums the 4 partials per batch.

Device schedule (fp16 matmul operands, fp32 PSUM):
- A1/A2: Q^T/K^T projections in [d, q] layout (d on partitions), m-outer so
  the PE chases the X DMAs; PSUM->SBUF copies on DVE so ACT stays exp-only.
- Attention is one global conveyor over units (i, hp, g, hl):
  ST (PE, two heads packed into 64-row PE groups via tile_position) ->
  exp (ACT) -> causal mask (gpsimd affine_select, diagonal units only) ->
  PV (PE; V carries a ones column so PSUM row DK accumulates the softmax
  denominator). PV lags ST by DEPTH units, and V-projection/out-projection
  matmuls are interleaved as PE filler, so the ACT-bound softmax never
  leaves the PE idle (keeps the HAM clock warm).
- Per-head normalization starts the moment that head's PV finishes:
  denominator row -> reciprocal_approx_fast (DVE) -> partition_broadcast
  (gpsimd) -> multiply (DVE) -> out-proj granules enqueued as filler.
"""

from collections import deque

import numpy as np

import concourse.mybir as mybir
import concourse.tile as tile
from concourse import bacc
from concourse.bass_utils import run_bass_kernel_spmd
from concourse._compat import get_trn_type

F32 = mybir.dt.float32
F32R = mybir.dt.float32r
F16 = mybir.dt.float16
AF = mybir.ActivationFunctionType
ALU = mybir.AluOpType

B, T, D, H = 2, 2048, 1024, 16
DK = 64
NCORES = 8
GROUPS = 4          # head-groups = cores per batch
DG = 256            # d-columns per core (4 heads x 64)
NH = 4              # heads per core
QB = 512            # query block
NQB = T // QB       # 4
KTILE = 128
NKT = T // KTILE    # 16
NMT = D // 128      # 8 contraction tiles over D
GK = 2              # k-tiles per conveyor unit
DEPTH = 4           # PV lag in conveyor units


def build_program():
    nc = bacc.Bacc(get_trn_type() or "TRN2", target_bir_lowering=False, debug=False)

    x1t = nc.dram_tensor("x1t", [D, T], F16, kind="ExternalInput").ap()
    x2t = nc.dram_tensor("x2t", [D, T], F16, kind="ExternalInput").ap()
    wq = nc.dram_tensor("wq", [128, NMT, DG], F16, kind="ExternalInput").ap()
    wk = nc.dram_tensor("wk", [128, NMT, DG], F16, kind="ExternalInput").ap()
    wv = nc.dram_tensor("wv", [128, NMT, DG], F16, kind="ExternalInput").ap()
    wo = nc.dram_tensor("wo", [128, 2, D], F16, kind="ExternalInput").ap()
    out = nc.dram_tensor("out", [T, D], F16, kind="ExternalOutput").ap()

    with tile.TileContext(nc) as tc:
        _emit(nc, tc, x1t, x2t, wq, wk, wv, wo, out)
    nc.compile()
    return nc


def _emit(nc, tc, x1t, x2t, wq, wk, wv, wo, out):
    from contextlib import ExitStack

    with ExitStack() as ctx:
        wpool = ctx.enter_context(tc.tile_pool(name="weights", bufs=1))
        qkv = ctx.enter_context(tc.tile_pool(name="qkv", bufs=1))

        # --- weights to SBUF (gpsimd queue; wq first: it gates the first MMs)
        wq_sb = wpool.tile([128, NMT, DG], F16)
        nc.gpsimd.dma_start(wq_sb[:], wq[:])
        wk_sb = wpool.tile([128, NMT, DG], F16)
        nc.gpsimd.dma_start(wk_sb[:], wk[:])
        wv_sb = wpool.tile([128, NMT, DG], F16)
        nc.gpsimd.dma_start(wv_sb[:], wv[:])
        wo_sb = wpool.tile([128, 2, D], F16)
        nc.gpsimd.dma_start(wo_sb[:], wo[:])

        # Residents: QT/KT as [128, dt, q]; V as per-j tiles [k, h, DK+1]
        qt_sb = qkv.tile([128, 2, T], F16)
        kt_sb = qkv.tile([128, 2, T], F16)
        v_sb = [qkv.tile([128, NH, DK + 1], F16, name=f"vsb{j}") for j in range(NKT)]
        for j in range(NKT):
            nc.gpsimd.memset(v_sb[j][:, :, DK : DK + 1], 1.0)

        # X2 resident (V-projection granules run inside the conveyor).
        # x2 DMAs go on the sync queue AFTER x1's so the two streams don't
        # halve each other's HBM bandwidth (A1 is gated by x1 alone).
        x2_sb = qkv.tile([128, NMT, T], F16)

        # ones row for the PE-side denominator broadcast (K=1 f16 matmul)
        ones_sb = wpool.tile([1, DK], F16)
        nc.vector.memset(ones_sb[:], 1.0)

        # --- A1: QT = WQ^T X1^T, m-outer so matmuls chase the x1 DMAs ---
        with tc.tile_pool(name="x1pool", bufs=1) as x1pool, tc.tile_pool(
            name="psA", bufs=1, space="PSUM"
        ) as psA:
            qps = [psA.tile([128, NQB, QB], F32, name=f"qps{t}") for t in range(2)]
            x1_sb = x1pool.tile([128, NMT, T], F16)
            for m in range(NMT):
                nc.sync.dma_start(x1_sb[:, m, :], x1t[m * 128 : (m + 1) * 128, :])
            for m in range(NMT):  # x2 queued behind all of x1
                nc.sync.dma_start(x2_sb[:, m, :], x2t[m * 128 : (m + 1) * 128, :])
            for m in range(NMT):
                for dt in range(2):
                    lhsT = wq_sb[:, m, dt * 128 : (dt + 1) * 128]
                    for qc in range(NQB):
                        nc.tensor.matmul(
                            qps[dt][:, qc, :],
                            lhsT,
                            x1_sb[:, m, qc * QB : (qc + 1) * QB],
                            start=(m == 0),
                            stop=(m == NMT - 1),
                        )
            for qc in range(NQB):
                for dt in range(2):
                    nc.vector.tensor_copy(
                        out=qt_sb[:, dt, qc * QB : (qc + 1) * QB],
                        in_=qps[dt][:, qc, :],
                    )

        # --- A2: KT (x2 resident by now) ---
        with tc.tile_pool(name="psK", bufs=1, space="PSUM") as psK:
            kps = [psK.tile([128, NQB, QB], F32, name=f"kps{t}") for t in range(2)]
            for m in range(NMT):
                for dt in range(2):
                    lhsT = wk_sb[:, m, dt * 128 : (dt + 1) * 128]
                    for kc in range(NQB):
                        nc.tensor.matmul(
                            kps[dt][:, kc, :],
                            lhsT,
                            x2_sb[:, m, kc * QB : (kc + 1) * QB],
                            start=(m == 0),
                            stop=(m == NMT - 1),
                        )
            for kc in range(NQB):
                for dt in range(2):
                    nc.vector.tensor_copy(
                        out=kt_sb[:, dt, kc * QB : (kc + 1) * QB],
                        in_=kps[dt][:, kc, :],
                    )

        # --- attention-era pools (PSUM: 4 + 2 + 1 + 1 = 8 banks) ---
        st_ps = ctx.enter_context(tc.tile_pool(name="st_ps", bufs=2, space="PSUM"))
        ct_ps = ctx.enter_context(tc.tile_pool(name="ct_ps", bufs=1, space="PSUM"))
        op_ps = ctx.enter_context(tc.tile_pool(name="op_ps", bufs=1, space="PSUM"))
        vp_ps = ctx.enter_context(tc.tile_pool(name="vp_ps", bufs=1, space="PSUM"))
        pt_pool = ctx.enter_context(tc.tile_pool(name="pt", bufs=6))
        lin_pool = ctx.enter_context(tc.tile_pool(name="lin", bufs=4))
        ctu_pool = ctx.enter_context(tc.tile_pool(name="ctu", bufs=4))
        ctn_pool = ctx.enter_context(tc.tile_pool(name="ctn", bufs=2))
        osb_pool = ctx.enter_context(tc.tile_pool(name="osb", bufs=2))

        # ---- V-projection granules (two 4-MM halves per k-tile j) ----
        vp_state = {}

        def vproj_half(j, half):
            if half == 0:
                vp_state[j] = vp_ps.tile([128, QB], F32, name="vps")
            vps = vp_state[j]
            for m in range(half * 4, half * 4 + 4):
                nc.tensor.matmul(
                    vps[:, 0:DG],
                    x2_sb[:, m, j * 128 : (j + 1) * 128],
                    wv_sb[:, m, :],
                    start=(m == 0),
                    stop=(m == NMT - 1),
                )
            if half == 1:
                for h in range(NH):
                    nc.vector.tensor_copy(
                        out=v_sb[j][:, h, 0:DK], in_=vps[:, h * DK : (h + 1) * DK]
                    )
                del vp_state[j]

        for j in range(4):  # needed by block i=0, before the conveyor
            vproj_half(j, 0)
            vproj_half(j, 1)

        # ---- out-projection granules (2 accumulating MMs + copy + DMA);
        # granules alternate between the op and vproj PSUM banks so
        # consecutive granules double-buffer without an extra bank ----
        op_count = [0]

        def outproj_granule(i, ctn_sb, qs, nch):
            op_count[0] += 1
            if op_count[0] % 2 == 0:
                ops = op_ps.tile([128, QB], F32, name="ops")
            else:
                ops = vp_ps.tile([128, QB], F32, name="vps")
            for dt in range(2):
                nc.tensor.matmul(
                    ops[:],
                    ctn_sb[:, dt, qs * 128 : (qs + 1) * 128],
                    wo_sb[:, dt, nch * QB : (nch + 1) * QB],
                    start=(dt == 0),
                    stop=(dt == 1),
                )
            osb = osb_pool.tile([128, QB], F16, name="osb")
            nc.vector.tensor_copy(out=osb[:], in_=ops[:])
            nc.sync.dma_start(
                out[
                    i * QB + qs * 128 : i * QB + (qs + 1) * 128,
                    nch * QB : (nch + 1) * QB,
                ],
                osb[:],
            )

        # ---- conveyor state ----
        fillers = deque()
        pend = deque()
        normq = deque()  # (enqueue_unit_idx, closure) - run >=2 units later
        ctp_cur = {}   # hp -> [ctp_hl0, ctp_hl1] for the active (i, hp)
        ctn_cur = {}   # i -> ctn tile

        def drain_norm(i, hp, hl, ctp, uidx):
            """Head (2*hp+hl) of block i: reciprocal of the denominator row,
            then (deferred) PE-broadcast into rows 64:128 of the same PSUM
            bank and a fused PSUM-read multiply into ctn."""
            if i not in ctn_cur:
                ctn_cur[i] = ctn_pool.tile([128, 2, QB], F16, name="ctn")
            ctn_sb = ctn_cur[i]
            lrow = lin_pool.tile([1, QB], F32, name="lrow")
            nc.vector.tensor_copy(out=lrow[:], in_=ctp[DK : DK + 1, :])
            linv = lin_pool.tile([1, QB], F32, name="linv")
            # NB: approx_fast misreads PSUM operands on HW - keep in_ in SBUF
            nc.vector.reciprocal_approx_fast(out=linv[:], in_=lrow[:])
            linv16 = lin_pool.tile([1, QB], F16, name="linv16")
            nc.vector.tensor_copy(out=linv16[:], in_=linv[:])
            ctu = ctu_pool.tile([DK, QB], F32, name="ctu")
            nc.vector.tensor_copy(out=ctu[:], in_=ctp[0:DK, :])

            def norm():
                # broadcast 1/denom across partitions via K=1 f16 matmul
                # into the upper half of the ctp bank (one PSUM operand max
                # per DVE op, so ctx was copied to SBUF above)
                nc.tensor.matmul(
                    ctp[DK : DK + DK, :],
                    ones_sb[:],
                    linv16[:],
                    start=True,
                    stop=True,
                    tile_position=(0, 64),
                )
                nc.vector.tensor_tensor(
                    ctn_sb[hl * DK : (hl + 1) * DK, hp, :],
                    ctu[:],
                    ctp[DK : DK + DK, :],
                    ALU.mult,
                )
                if hp == 1 and hl == 1:  # block i fully normalized -> out-proj
                    for qs in range(4):
                        for nch in range(2):
                            fillers.append(
                                lambda i=i, c=ctn_sb, qs=qs, nch=nch: outproj_granule(
                                    i, c, qs, nch
                                )
                            )

            normq.append((uidx, norm))

        def emit_pv(u, pt, uidx):
            i, hp, g, hl = u
            njt = 4 * (i + 1)
            if g == 0 and hl == 0:
                ctp_cur[hp] = [
                    ct_ps.tile([128, QB], F32, name=f"ctp{t}") for t in range(2)
                ]
            ctp = ctp_cur[hp][hl]
            for jj in range(GK):
                j = g * GK + jj
                nc.tensor.matmul(
                    ctp[0 : DK + 1, :],
                    v_sb[j][:, hp * 2 + hl, :],
                    pt[:, jj, :],
                    start=(j == 0),
                    stop=(j == njt - 1),
                )
            if g == 2 * i + 1:  # this head's last PV -> normalize
                drain_norm(i, hp, hl, ctp, uidx)

        units = []
        for i in range(NQB):
            for hp in range(2):
                for g in range(2 * i + 2):
                    for hl in range(2):
                        units.append((i, hp, g, hl))

        cur_block = -1
        for uidx, u in enumerate(units):
            i, hp, g, hl = u
            if i != cur_block:
                cur_block = i
                # V-projection for the NEXT block's k-tiles rides as filler
                for j in range(4 * (i + 1), min(4 * (i + 2), NKT)):
                    fillers.append(lambda j=j: vproj_half(j, 0))
                    fillers.append(lambda j=j: vproj_half(j, 1))
            # ST first: the PE instruction in front is always the one the
            # ACT stream is waiting on, so exp runs gapless; PV/fillers
            # execute behind it during the exp itself.
            lo, hi = hl * 64, hl * 64 + 64
            stm = st_ps.tile([128, GK, QB], F32, name="stm")
            for jj in range(GK):
                j = g * GK + jj
                nc.tensor.matmul(
                    stm[:, jj, :],
                    kt_sb[lo:hi, hp, j * 128 : (j + 1) * 128],
                    qt_sb[lo:hi, hp, i * QB : (i + 1) * QB],
                    start=True,
                    stop=True,
                    tile_position=(lo, 0),
                )
            pt = pt_pool.tile([128, GK, QB], F16)
            trim = max(0, 128 * (g * GK - 4 * i))  # no q < trim is unmasked
            if trim > 0:
                # exp only the potentially-valid q range; affine_select
                # below zero-fills the whole tile wherever its predicate
                # is false, which covers the untouched q < trim region.
                nc.scalar.activation(
                    pt[:, :, trim:QB], stm[:, :, trim:QB], AF.Exp
                )
            else:
                nc.scalar.activation(pt[:], stm[:], AF.Exp)
            if g * GK >= 4 * i:
                # diagonal group: keep (p, jj, f) iff
                # f - p - 128*(g*GK - 4i) - 128*jj >= 0
                nc.gpsimd.affine_select(
                    pt[:],
                    pt[:],
                    pattern=[[-128, GK], [1, QB]],
                    compare_op=ALU.is_ge,
                    fill=0.0,
                    base=-(128 * (g * GK - 4 * i)),
                    channel_multiplier=-1,
                )
            pend.append((u, pt))
            if len(pend) > DEPTH:
                emit_pv(*pend.popleft(), uidx)
            if normq and normq[0][0] <= uidx - 2:
                normq.popleft()[1]()
            if fillers:
                fillers.popleft()()

        uidx = len(units)
        while pend:
            emit_pv(*pend.popleft(), uidx)
            uidx += 1
        while normq:
            normq.popleft()[1]()
        while fillers:  # out-proj of the last block
            fillers.popleft()()


_PROG = None


def _get_prog():
    global _PROG
    if _PROG is None:
        _PROG = build_program()
    return _PROG


def _wlayout(w):
    """[(n_out*128), f] -> [128, n_out, f] contiguous fp16 (device lhsT layout)."""
    n_out = w.shape[0] // 128
    return np.ascontiguousarray(
        w.reshape(n_out, 128, w.shape[1]).transpose(1, 0, 2)
    ).astype(np.float16)


def shard_inputs(X1, X2, WQ, WK, WV, WO):
    X1 = np.asarray(X1, dtype=np.float32)
    X2 = np.asarray(X2, dtype=np.float32)
    WQ = np.asarray(WQ, dtype=np.float32)
    WK = np.asarray(WK, dtype=np.float32)
    WV = np.asarray(WV, dtype=np.float32)
    WO = np.asarray(WO, dtype=np.float32)
    x1t = [np.ascontiguousarray(X1[b].T).astype(np.float16) for b in range(B)]
    x2t = [np.ascontiguousarray(X2[b].T).astype(np.float16) for b in range(B)]
    in_maps = []
    for c in range(NCORES):
        b, g = divmod(c, GROUPS)
        sl = slice(g * DG, (g + 1) * DG)
        in_maps.append(
            {
                "x1t": x1t[b],
                "x2t": x2t[b],
                # fold the 1/sqrt(DK) score scale into WQ (exact: power of 2)
                "wq": _wlayout(WQ[:, sl] * np.float32(0.125)),
                "wk": _wlayout(WK[:, sl]),
                "wv": _wlayout(WV[:, sl]),
                "wo": _wlayout(WO[sl, :]),
            }
        )
    return in_maps


LAST_RESULT = None


def kernel(X1, X2, padding_mask, WQ, WK, WV, WO, **kwargs):
    global LAST_RESULT
    del padding_mask  # all-False by construction (spec fill: zeros)
    nc = _get_prog()
    in_maps = shard_inputs(X1, X2, WQ, WK, WV, WO)
    res = run_bass_kernel_spmd(nc, in_maps, list(range(NCORES)), **kwargs)
    LAST_RESULT = res
    out = np.zeros((B, T, D), dtype=np.float32)
    for c in range(NCORES):
        out[c // GROUPS] += res.results[c]["out"]
    return out


# revision 26
# speedup vs baseline: 1.1555x; 1.0637x over previous
"""Trainium2 Bass kernel for multi-head attention (B=2, T=2048, D=1024, H=16).

Sharding (Megatron-style): 8 cores = 2 batches x 4 head-groups. Core c handles
batch c//4 and heads [4*(c%4), 4*(c%4)+4): WQ/WK/WV split column-wise (256
cols per core), WO split row-wise. Each core writes a (T, D) fp16 partial
output; the host sums the 4 partials per batch.

Device schedule (fp16 matmul operands, fp32 PSUM):
- A1/A2: Q^T/K^T projections in [d, q] layout (d on partitions), m-outer so
  the PE chases the X DMAs; PSUM->SBUF copies on DVE so ACT stays exp-only.
- Attention is one global conveyor over units (i, hp, g, hl):
  ST (PE, two heads packed into 64-row PE groups via tile_position) ->
  exp (ACT) -> causal mask (gpsimd affine_select, diagonal units only) ->
  PV (PE; V carries a ones column so PSUM row DK accumulates the softmax
  denominator). PV lags ST by DEPTH units, and V-projection/out-projection
  matmuls are interleaved as PE filler, so the ACT-bound softmax never
  leaves the PE idle (keeps the HAM clock warm).
- Per-head normalization starts the moment that head's PV finishes:
  denominator row -> reciprocal_approx_fast (DVE) -> partition_broadcast
  (gpsimd) -> multiply (DVE) -> out-proj granules enqueued as filler.
"""

from collections import deque

import numpy as np

import concourse.mybir as mybir
import concourse.tile as tile
from concourse import bacc
from concourse.bass_utils import run_bass_kernel_spmd
from concourse._compat import get_trn_type

F32 = mybir.dt.float32
F32R = mybir.dt.float32r
F16 = mybir.dt.float16
AF = mybir.ActivationFunctionType
ALU = mybir.AluOpType

B, T, D, H = 2, 2048, 1024, 16
DK = 64
NCORES = 8
GROUPS = 4          # head-groups = cores per batch
DG = 256            # d-columns per core (4 heads x 64)
NH = 4              # heads per core
QB = 512            # query block
NQB = T // QB       # 4
KTILE = 128
NKT = T // KTILE    # 16
NMT = D // 128      # 8 contraction tiles over D
GK = 2              # k-tiles per conveyor unit
DEPTH = 4           # PV lag in conveyor units


def build_program():
    nc = bacc.Bacc(get_trn_type() or "TRN2", target_bir_lowering=False, debug=False)

    x1t = nc.dram_tensor("x1t", [D, T], F16, kind="ExternalInput").ap()
    x2t = nc.dram_tensor("x2t", [D, T], F16, kind="ExternalInput").ap()
    wq = nc.dram_tensor("wq", [128, NMT, DG], F16, kind="ExternalInput").ap()
    wk = nc.dram_tensor("wk", [128, NMT, DG], F16, kind="ExternalInput").ap()
    wv = nc.dram_tensor("wv", [128, NMT, DG], F16, kind="ExternalInput").ap()
    wo = nc.dram_tensor("wo", [128, 2, D], F16, kind="ExternalInput").ap()
    out = nc.dram_tensor("out", [T, D], F16, kind="ExternalOutput").ap()

    with tile.TileContext(nc) as tc:
        _emit(nc, tc, x1t, x2t, wq, wk, wv, wo, out)
    nc.compile()
    return nc


def _emit(nc, tc, x1t, x2t, wq, wk, wv, wo, out):
    from contextlib import ExitStack

    with ExitStack() as ctx:
        wpool = ctx.enter_context(tc.tile_pool(name="weights", bufs=1))
        qkv = ctx.enter_context(tc.tile_pool(name="qkv", bufs=1))

        # --- weights to SBUF (gpsimd queue; wq first: it gates the first MMs)
        wq_sb = wpool.tile([128, NMT, DG], F16)
        nc.gpsimd.dma_start(wq_sb[:], wq[:])
        wk_sb = wpool.tile([128, NMT, DG], F16)
        nc.gpsimd.dma_start(wk_sb[:], wk[:])
        wv_sb = wpool.tile([128, NMT, DG], F16)
        nc.gpsimd.dma_start(wv_sb[:], wv[:])
        wo_sb = wpool.tile([128, 2, D], F16)
        nc.gpsimd.dma_start(wo_sb[:], wo[:])

        # Residents: QT/KT as [128, dt, q]; V as per-j tiles [k, h, DK+1]
        qt_sb = qkv.tile([128, 2, T], F16)
        kt_sb = qkv.tile([128, 2, T], F16)
        v_sb = [qkv.tile([128, NH, DK + 1], F16, name=f"vsb{j}") for j in range(NKT)]
        for j in range(NKT):
            nc.gpsimd.memset(v_sb[j][:, :, DK : DK + 1], 1.0)

        # X2 resident (V-projection granules run inside the conveyor).
        # x2 DMAs go on the sync queue AFTER x1's so the two streams don't
        # halve each other's HBM bandwidth (A1 is gated by x1 alone).
        x2_sb = qkv.tile([128, NMT, T], F16)

        # ones row for the PE-side denominator broadcast (K=1 f16 matmul)
        ones_sb = wpool.tile([1, DK], F16)
        nc.vector.memset(ones_sb[:], 1.0)

        # --- A1: QT = WQ^T X1^T, m-outer so matmuls chase the x1 DMAs ---
        with tc.tile_pool(name="x1pool", bufs=1) as x1pool, tc.tile_pool(
            name="psA", bufs=1, space="PSUM"
        ) as psA:
            qps = [psA.tile([128, NQB, QB], F32, name=f"qps{t}") for t in range(2)]
            x1_sb = x1pool.tile([128, NMT, T], F16)
            for m in range(NMT):
                nc.sync.dma_start(x1_sb[:, m, :], x1t[m * 128 : (m + 1) * 128, :])
            for m in range(NMT):  # x2 queued behind all of x1
                nc.sync.dma_start(x2_sb[:, m, :], x2t[m * 128 : (m + 1) * 128, :])
            for m in range(NMT):
                for dt in range(2):
                    lhsT = wq_sb[:, m, dt * 128 : (dt + 1) * 128]
                    for qc in range(NQB):
                        nc.tensor.matmul(
                            qps[dt][:, qc, :],
                            lhsT,
                            x1_sb[:, m, qc * QB : (qc + 1) * QB],
                            start=(m == 0),
                            stop=(m == NMT - 1),
                        )
                    if m == NMT - 1:
                        # drain this dt's PSUM immediately (ACT+DVE split)
                        # so the next pool's bank WAR clears during the
                        # remaining matmuls instead of after them
                        for qc in range(NQB):
                            if qc % 2 == 0:
                                nc.scalar.copy(
                                    qt_sb[:, dt, qc * QB : (qc + 1) * QB],
                                    qps[dt][:, qc, :],
                                )
                            else:
                                nc.vector.tensor_copy(
                                    out=qt_sb[:, dt, qc * QB : (qc + 1) * QB],
                                    in_=qps[dt][:, qc, :],
                                )

        # --- A2: KT (x2 resident by now) ---
        with tc.tile_pool(name="psK", bufs=1, space="PSUM") as psK:
            kps = [psK.tile([128, NQB, QB], F32, name=f"kps{t}") for t in range(2)]
            for m in range(NMT):
                for dt in range(2):
                    lhsT = wk_sb[:, m, dt * 128 : (dt + 1) * 128]
                    for kc in range(NQB):
                        nc.tensor.matmul(
                            kps[dt][:, kc, :],
                            lhsT,
                            x2_sb[:, m, kc * QB : (kc + 1) * QB],
                            start=(m == 0),
                            stop=(m == NMT - 1),
                        )
                    if m == NMT - 1:
                        for kc in range(NQB):
                            if kc % 2 == 0:
                                nc.scalar.copy(
                                    kt_sb[:, dt, kc * QB : (kc + 1) * QB],
                                    kps[dt][:, kc, :],
                                )
                            else:
                                nc.vector.tensor_copy(
                                    out=kt_sb[:, dt, kc * QB : (kc + 1) * QB],
                                    in_=kps[dt][:, kc, :],
                                )

        # --- attention-era pools (PSUM: 4 + 2 + 1 + 1 = 8 banks) ---
        st_ps = ctx.enter_context(tc.tile_pool(name="st_ps", bufs=2, space="PSUM"))
        ct_ps = ctx.enter_context(tc.tile_pool(name="ct_ps", bufs=1, space="PSUM"))
        op_ps = ctx.enter_context(tc.tile_pool(name="op_ps", bufs=1, space="PSUM"))
        vp_ps = ctx.enter_context(tc.tile_pool(name="vp_ps", bufs=1, space="PSUM"))
        pt_pool = ctx.enter_context(tc.tile_pool(name="pt", bufs=6))
        lin_pool = ctx.enter_context(tc.tile_pool(name="lin", bufs=4))
        ctu_pool = ctx.enter_context(tc.tile_pool(name="ctu", bufs=4))
        ctn_pool = ctx.enter_context(tc.tile_pool(name="ctn", bufs=2))
        osb_pool = ctx.enter_context(tc.tile_pool(name="osb", bufs=3))

        # ---- V-projection granules (two 4-MM halves per k-tile j) ----
        vp_state = {}

        def vproj_half(j, half):
            if half == 0:
                vp_state[j] = vp_ps.tile([128, QB], F32, name="vps")
            vps = vp_state[j]
            for m in range(half * 4, half * 4 + 4):
                nc.tensor.matmul(
                    vps[:, 0:DG],
                    x2_sb[:, m, j * 128 : (j + 1) * 128],
                    wv_sb[:, m, :],
                    start=(m == 0),
                    stop=(m == NMT - 1),
                )
            if half == 1:
                for h in range(NH):
                    nc.vector.tensor_copy(
                        out=v_sb[j][:, h, 0:DK], in_=vps[:, h * DK : (h + 1) * DK]
                    )
                del vp_state[j]

        for j in range(4):  # needed by block i=0, before the conveyor
            vproj_half(j, 0)
            vproj_half(j, 1)

        # ---- out-projection granules (2 accumulating MMs + copy + DMA);
        # granules alternate between the op and vproj PSUM banks so
        # consecutive granules double-buffer without an extra bank ----
        op_count = [0]

        def outproj_granule(i, ctn_sb, qs, nch):
            op_count[0] += 1
            if op_count[0] % 2 == 0:
                ops = op_ps.tile([128, QB], F32, name="ops")
            else:
                ops = vp_ps.tile([128, QB], F32, name="vps")
            for dt in range(2):
                nc.tensor.matmul(
                    ops[:],
                    ctn_sb[:, dt, qs * 128 : (qs + 1) * 128],
                    wo_sb[:, dt, nch * QB : (nch + 1) * QB],
                    start=(dt == 0),
                    stop=(dt == 1),
                )
            osb = osb_pool.tile([128, QB], F16, name="osb")
            nc.vector.tensor_copy(out=osb[:], in_=ops[:])
            nc.sync.dma_start(
                out[
                    i * QB + qs * 128 : i * QB + (qs + 1) * 128,
                    nch * QB : (nch + 1) * QB,
                ],
                osb[:],
            )

        # ---- conveyor state ----
        fillers = deque()
        pend = deque()
        normq = deque()  # (enqueue_unit_idx, closure) - run >=2 units later
        ctp_cur = {}   # hp -> [ctp_hl0, ctp_hl1] for the active (i, hp)
        ctn_cur = {}   # i -> ctn tile

        def drain_norm(i, hp, hl, ctp, uidx):
            """Head (2*hp+hl) of block i: reciprocal of the denominator row,
            then (deferred) PE-broadcast into rows 64:128 of the same PSUM
            bank and a fused PSUM-read multiply into ctn."""
            if i not in ctn_cur:
                ctn_cur[i] = ctn_pool.tile([128, 2, QB], F16, name="ctn")
            ctn_sb = ctn_cur[i]
            lrow = lin_pool.tile([1, QB], F32, name="lrow")
            nc.vector.tensor_copy(out=lrow[:], in_=ctp[DK : DK + 1, :])
            linv = lin_pool.tile([1, QB], F32, name="linv")
            # NB: approx_fast misreads PSUM operands on HW - keep in_ in SBUF
            nc.vector.reciprocal_approx_fast(out=linv[:], in_=lrow[:])
            linv16 = lin_pool.tile([1, QB], F16, name="linv16")
            nc.vector.tensor_copy(out=linv16[:], in_=linv[:])
            ctu = ctu_pool.tile([DK, QB], F32, name="ctu")
            nc.vector.tensor_copy(out=ctu[:], in_=ctp[0:DK, :])

            def norm():
                # broadcast 1/denom across partitions via K=1 f16 matmul
                # into the upper half of the ctp bank (one PSUM operand max
                # per DVE op, so ctx was copied to SBUF above)
                nc.tensor.matmul(
                    ctp[DK : DK + DK, :],
                    ones_sb[:],
                    linv16[:],
                    start=True,
                    stop=True,
                    tile_position=(0, 64),
                )
                nc.vector.tensor_tensor(
                    ctn_sb[hl * DK : (hl + 1) * DK, hp, :],
                    ctu[:],
                    ctp[DK : DK + DK, :],
                    ALU.mult,
                )
                if hp == 1 and hl == 1:  # block i fully normalized -> out-proj
                    for qs in range(4):
                        for nch in range(2):
                            fillers.append(
                                lambda i=i, c=ctn_sb, qs=qs, nch=nch: outproj_granule(
                                    i, c, qs, nch
                                )
                            )

            normq.append((uidx, norm))

        def emit_pv(u, pt, uidx):
            i, hp, g, hl = u
            njt = 4 * (i + 1)
            if g == 0 and hl == 0:
                ctp_cur[hp] = [
                    ct_ps.tile([128, QB], F32, name=f"ctp{t}") for t in range(2)
                ]
            ctp = ctp_cur[hp][hl]
            for jj in range(GK):
                j = g * GK + jj
                nc.tensor.matmul(
                    ctp[0 : DK + 1, :],
                    v_sb[j][:, hp * 2 + hl, :],
                    pt[:, jj, :],
                    start=(j == 0),
                    stop=(j == njt - 1),
                )
            if g == 2 * i + 1:  # this head's last PV -> normalize
                drain_norm(i, hp, hl, ctp, uidx)

        units = []
        for i in range(NQB):
            for hp in range(2):
                for g in range(2 * i + 2):
                    for hl in range(2):
                        units.append((i, hp, g, hl))

        cur_block = -1
        for uidx, u in enumerate(units):
            i, hp, g, hl = u
            if i != cur_block:
                cur_block = i
                # V-projection for the NEXT block's k-tiles rides as filler
                for j in range(4 * (i + 1), min(4 * (i + 2), NKT)):
                    fillers.append(lambda j=j: vproj_half(j, 0))
                    fillers.append(lambda j=j: vproj_half(j, 1))
            # ST first: the PE instruction in front is always the one the
            # ACT stream is waiting on, so exp runs gapless; PV/fillers
            # execute behind it during the exp itself.
            lo, hi = hl * 64, hl * 64 + 64
            stm = st_ps.tile([128, GK, QB], F32, name="stm")
            for jj in range(GK):
                j = g * GK + jj
                nc.tensor.matmul(
                    stm[:, jj, :],
                    kt_sb[lo:hi, hp, j * 128 : (j + 1) * 128],
                    qt_sb[lo:hi, hp, i * QB : (i + 1) * QB],
                    start=True,
                    stop=True,
                    tile_position=(lo, 0),
                )
            pt = pt_pool.tile([128, GK, QB], F16)
            trim = max(0, 128 * (g * GK - 4 * i))  # no q < trim is unmasked
            if trim > 0:
                # exp only the potentially-valid q range; affine_select
                # below zero-fills the whole tile wherever its predicate
                # is false, which covers the untouched q < trim region.
                nc.scalar.activation(
                    pt[:, :, trim:QB], stm[:, :, trim:QB], AF.Exp
                )
            else:
                nc.scalar.activation(pt[:], stm[:], AF.Exp)
            if g * GK >= 4 * i:
                # diagonal group: keep (p, jj, f) iff
                # f - p - 128*(g*GK - 4i) - 128*jj >= 0
                nc.gpsimd.affine_select(
                    pt[:],
                    pt[:],
                    pattern=[[-128, GK], [1, QB]],
                    compare_op=ALU.is_ge,
                    fill=0.0,
                    base=-(128 * (g * GK - 4 * i)),
                    channel_multiplier=-1,
                )
            pend.append((u, pt))
            if len(pend) > DEPTH:
                emit_pv(*pend.popleft(), uidx)
            if normq and normq[0][0] <= uidx - 2:
                normq.popleft()[1]()
            if fillers:
                fillers.popleft()()

        uidx = len(units)
        while pend:
            emit_pv(*pend.popleft(), uidx)
            uidx += 1
        while normq:
            normq.popleft()[1]()
        while fillers:  # out-proj of the last block
            fillers.popleft()()


_PROG = None


def _get_prog():
    global _PROG
    if _PROG is None:
        _PROG = build_program()
    return _PROG


def _wlayout(w):
    """[(n_out*128), f] -> [128, n_out, f] contiguous fp16 (device lhsT layout)."""
    n_out = w.shape[0] // 128
    return np.ascontiguousarray(
        w.reshape(n_out, 128, w.shape[1]).transpose(1, 0, 2)
    ).astype(np.float16)


def shard_inputs(X1, X2, WQ, WK, WV, WO):
    X1 = np.asarray(X1, dtype=np.float32)
    X2 = np.asarray(X2, dtype=np.float32)
    WQ = np.asarray(WQ, dtype=np.float32)
    WK = np.asarray(WK, dtype=np.float32)
    WV = np.asarray(WV, dtype=np.float32)
    WO = np.asarray(WO, dtype=np.float32)
    x1t = [np.ascontiguousarray(X1[b].T).astype(np.float16) for b in range(B)]
    x2t = [np.ascontiguousarray(X2[b].T).astype(np.float16) for b in range(B)]
    in_maps = []
    for c in range(NCORES):
        b, g = divmod(c, GROUPS)
        sl = slice(g * DG, (g + 1) * DG)
        in_maps.append(
            {
                "x1t": x1t[b],
                "x2t": x2t[b],
                # fold the 1/sqrt(DK) score scale into WQ (exact: power of 2)
                "wq": _wlayout(WQ[:, sl] * np.float32(0.125)),
                "wk": _wlayout(WK[:, sl]),
                "wv": _wlayout(WV[:, sl]),
                "wo": _wlayout(WO[sl, :]),
            }
        )
    return in_maps


LAST_RESULT = None


def kernel(X1, X2, padding_mask, WQ, WK, WV, WO, **kwargs):
    global LAST_RESULT
    del padding_mask  # all-False by construction (spec fill: zeros)
    nc = _get_prog()
    in_maps = shard_inputs(X1, X2, WQ, WK, WV, WO)
    res = run_bass_kernel_spmd(nc, in_maps, list(range(NCORES)), **kwargs)
    LAST_RESULT = res
    out = np.zeros((B, T, D), dtype=np.float32)
    for c in range(NCORES):
        out[c // GROUPS] += res.results[c]["out"]
    return out


# revision 28
# speedup vs baseline: 1.1587x; 1.0027x over previous
"""Trainium2 Bass kernel for multi-head attention (B=2, T=2048, D=1024, H=16).

Sharding (Megatron-style): 8 cores = 2 batches x 4 head-groups. Core c handles
batch c//4 and heads [4*(c%4), 4*(c%4)+4): WQ/WK/WV split column-wise (256
cols per core), WO split row-wise. Each core writes a (T, D) fp16 partial
output; the host sums the 4 partials per batch.

Device schedule (fp16 matmul operands, fp32 PSUM):
- A1/A2: Q^T/K^T projections in [d, q] layout (d on partitions), m-outer so
  the PE chases the X DMAs; PSUM->SBUF copies on DVE so ACT stays exp-only.
- Attention is one global conveyor over units (i, hp, g, hl):
  ST (PE, two heads packed into 64-row PE groups via tile_position) ->
  exp (ACT) -> causal mask (gpsimd affine_select, diagonal units only) ->
  PV (PE; V carries a ones column so PSUM row DK accumulates the softmax
  denominator). PV lags ST by DEPTH units, and V-projection/out-projection
  matmuls are interleaved as PE filler, so the ACT-bound softmax never
  leaves the PE idle (keeps the HAM clock warm).
- Per-head normalization starts the moment that head's PV finishes:
  denominator row -> reciprocal_approx_fast (DVE) -> partition_broadcast
  (gpsimd) -> multiply (DVE) -> out-proj granules enqueued as filler.
"""

from collections import deque

import numpy as np

import concourse.mybir as mybir
import concourse.tile as tile
from concourse import bacc
from concourse.bass_utils import run_bass_kernel_spmd
from concourse._compat import get_trn_type

F32 = mybir.dt.float32
F32R = mybir.dt.float32r
F16 = mybir.dt.float16
AF = mybir.ActivationFunctionType
ALU = mybir.AluOpType

B, T, D, H = 2, 2048, 1024, 16
DK = 64
NCORES = 8
GROUPS = 4          # head-groups = cores per batch
DG = 256            # d-columns per core (4 heads x 64)
NH = 4              # heads per core
QB = 512            # query block
NQB = T // QB       # 4
KTILE = 128
NKT = T // KTILE    # 16
NMT = D // 128      # 8 contraction tiles over D
GK = 2              # k-tiles per conveyor unit
DEPTH = 4           # PV lag in conveyor units


def build_program():
    nc = bacc.Bacc(get_trn_type() or "TRN2", target_bir_lowering=False, debug=False)

    x1t = nc.dram_tensor("x1t", [D, T], F16, kind="ExternalInput").ap()
    x2t = nc.dram_tensor("x2t", [D, T], F16, kind="ExternalInput").ap()
    wq = nc.dram_tensor("wq", [128, NMT, DG], F16, kind="ExternalInput").ap()
    wk = nc.dram_tensor("wk", [128, NMT, DG], F16, kind="ExternalInput").ap()
    wv = nc.dram_tensor("wv", [128, NMT, DG], F16, kind="ExternalInput").ap()
    wo = nc.dram_tensor("wo", [128, 2, D], F16, kind="ExternalInput").ap()
    out = nc.dram_tensor("out", [T, D], F16, kind="ExternalOutput").ap()

    with tile.TileContext(nc) as tc:
        _emit(nc, tc, x1t, x2t, wq, wk, wv, wo, out)
    nc.compile()
    return nc


def _emit(nc, tc, x1t, x2t, wq, wk, wv, wo, out):
    from contextlib import ExitStack

    with ExitStack() as ctx:
        wpool = ctx.enter_context(tc.tile_pool(name="weights", bufs=1))
        qkv = ctx.enter_context(tc.tile_pool(name="qkv", bufs=1))

        # --- weights to SBUF (gpsimd queue; wq first: it gates the first MMs)
        wq_sb = wpool.tile([128, NMT, DG], F16)
        nc.gpsimd.dma_start(wq_sb[:], wq[:])
        wk_sb = wpool.tile([128, NMT, DG], F16)
        nc.gpsimd.dma_start(wk_sb[:], wk[:])
        wv_sb = wpool.tile([128, NMT, DG], F16)
        nc.gpsimd.dma_start(wv_sb[:], wv[:])
        wo_sb = wpool.tile([128, 2, D], F16)
        nc.gpsimd.dma_start(wo_sb[:], wo[:])

        # Residents: QT/KT as [128, dt, q]; V as per-j tiles [k, h, DK+1]
        qt_sb = qkv.tile([128, 2, T], F16)
        kt_sb = qkv.tile([128, 2, T], F16)
        v_sb = [qkv.tile([128, NH, DK + 1], F16, name=f"vsb{j}") for j in range(NKT)]
        for j in range(NKT):
            nc.gpsimd.memset(v_sb[j][:, :, DK : DK + 1], 1.0)

        # X2 resident (V-projection granules run inside the conveyor).
        # x2 DMAs go on the sync queue AFTER x1's so the two streams don't
        # halve each other's HBM bandwidth (A1 is gated by x1 alone).
        x2_sb = qkv.tile([128, NMT, T], F16)

        # ones row for the PE-side denominator broadcast (K=1 f16 matmul)
        ones_sb = wpool.tile([1, DK], F16)
        nc.vector.memset(ones_sb[:], 1.0)

        # --- A1: QT = WQ^T X1^T, m-outer so matmuls chase the x1 DMAs ---
        with tc.tile_pool(name="x1pool", bufs=1) as x1pool, tc.tile_pool(
            name="psA", bufs=1, space="PSUM"
        ) as psA:
            qps = [psA.tile([128, NQB, QB], F32, name=f"qps{t}") for t in range(2)]
            x1_sb = x1pool.tile([128, NMT, T], F16)
            for m in range(NMT):
                nc.sync.dma_start(x1_sb[:, m, :], x1t[m * 128 : (m + 1) * 128, :])
            for m in range(NMT):  # x2 queued behind all of x1
                nc.sync.dma_start(x2_sb[:, m, :], x2t[m * 128 : (m + 1) * 128, :])
            # dt-outer: dt0's PSUM drains (ACT+DVE split) while dt1's
            # matmuls run, so the banks recycle into the next pool early
            for dt in range(2):
                for m in range(NMT):
                    lhsT = wq_sb[:, m, dt * 128 : (dt + 1) * 128]
                    for qc in range(NQB):
                        nc.tensor.matmul(
                            qps[dt][:, qc, :],
                            lhsT,
                            x1_sb[:, m, qc * QB : (qc + 1) * QB],
                            start=(m == 0),
                            stop=(m == NMT - 1),
                        )
                for qc in range(NQB):
                    if qc % 2 == 0:
                        nc.scalar.copy(
                            qt_sb[:, dt, qc * QB : (qc + 1) * QB],
                            qps[dt][:, qc, :],
                        )
                    else:
                        nc.vector.tensor_copy(
                            out=qt_sb[:, dt, qc * QB : (qc + 1) * QB],
                            in_=qps[dt][:, qc, :],
                        )

        # --- A2: KT (x2 resident by now) ---
        with tc.tile_pool(name="psK", bufs=1, space="PSUM") as psK:
            kps = [psK.tile([128, NQB, QB], F32, name=f"kps{t}") for t in range(2)]
            for dt in range(2):
                for m in range(NMT):
                    lhsT = wk_sb[:, m, dt * 128 : (dt + 1) * 128]
                    for kc in range(NQB):
                        nc.tensor.matmul(
                            kps[dt][:, kc, :],
                            lhsT,
                            x2_sb[:, m, kc * QB : (kc + 1) * QB],
                            start=(m == 0),
                            stop=(m == NMT - 1),
                        )
                for kc in range(NQB):
                    if kc % 2 == 0:
                        nc.scalar.copy(
                            kt_sb[:, dt, kc * QB : (kc + 1) * QB],
                            kps[dt][:, kc, :],
                        )
                    else:
                        nc.vector.tensor_copy(
                            out=kt_sb[:, dt, kc * QB : (kc + 1) * QB],
                            in_=kps[dt][:, kc, :],
                        )

        # --- attention-era pools (PSUM: 4 + 2 + 1 + 1 = 8 banks) ---
        st_ps = ctx.enter_context(tc.tile_pool(name="st_ps", bufs=2, space="PSUM"))
        ct_ps = ctx.enter_context(tc.tile_pool(name="ct_ps", bufs=1, space="PSUM"))
        op_ps = ctx.enter_context(tc.tile_pool(name="op_ps", bufs=1, space="PSUM"))
        vp_ps = ctx.enter_context(tc.tile_pool(name="vp_ps", bufs=1, space="PSUM"))
        pt_pool = ctx.enter_context(tc.tile_pool(name="pt", bufs=6))
        lin_pool = ctx.enter_context(tc.tile_pool(name="lin", bufs=4))
        ctu_pool = ctx.enter_context(tc.tile_pool(name="ctu", bufs=4))
        ctn_pool = ctx.enter_context(tc.tile_pool(name="ctn", bufs=2))
        osb_pool = ctx.enter_context(tc.tile_pool(name="osb", bufs=3))

        # ---- V-projection granules (two 4-MM halves per k-tile j) ----
        vp_state = {}

        def vproj_half(j, half):
            if half == 0:
                vp_state[j] = vp_ps.tile([128, QB], F32, name="vps")
            vps = vp_state[j]
            for m in range(half * 4, half * 4 + 4):
                nc.tensor.matmul(
                    vps[:, 0:DG],
                    x2_sb[:, m, j * 128 : (j + 1) * 128],
                    wv_sb[:, m, :],
                    start=(m == 0),
                    stop=(m == NMT - 1),
                )
            if half == 1:
                for h in range(NH):
                    nc.vector.tensor_copy(
                        out=v_sb[j][:, h, 0:DK], in_=vps[:, h * DK : (h + 1) * DK]
                    )
                del vp_state[j]

        for j in range(4):  # needed by block i=0, before the conveyor
            vproj_half(j, 0)
            vproj_half(j, 1)

        # ---- out-projection granules (2 accumulating MMs + copy + DMA);
        # granules alternate between the op and vproj PSUM banks so
        # consecutive granules double-buffer without an extra bank ----
        op_count = [0]

        def outproj_granule(i, ctn_sb, qs, nch):
            op_count[0] += 1
            if op_count[0] % 2 == 0:
                ops = op_ps.tile([128, QB], F32, name="ops")
            else:
                ops = vp_ps.tile([128, QB], F32, name="vps")
            for dt in range(2):
                nc.tensor.matmul(
                    ops[:],
                    ctn_sb[:, dt, qs * 128 : (qs + 1) * 128],
                    wo_sb[:, dt, nch * QB : (nch + 1) * QB],
                    start=(dt == 0),
                    stop=(dt == 1),
                )
            osb = osb_pool.tile([128, QB], F16, name="osb")
            nc.vector.tensor_copy(out=osb[:], in_=ops[:])
            nc.sync.dma_start(
                out[
                    i * QB + qs * 128 : i * QB + (qs + 1) * 128,
                    nch * QB : (nch + 1) * QB,
                ],
                osb[:],
            )

        # ---- conveyor state ----
        fillers = deque()
        pend = deque()
        normq = deque()  # (enqueue_unit_idx, closure) - run >=2 units later
        ctp_cur = {}   # hp -> [ctp_hl0, ctp_hl1] for the active (i, hp)
        ctn_cur = {}   # i -> ctn tile

        def drain_norm(i, hp, hl, ctp, uidx):
            """Head (2*hp+hl) of block i: reciprocal of the denominator row,
            then (deferred) PE-broadcast into rows 64:128 of the same PSUM
            bank and a fused PSUM-read multiply into ctn."""
            if i not in ctn_cur:
                ctn_cur[i] = ctn_pool.tile([128, 2, QB], F16, name="ctn")
            ctn_sb = ctn_cur[i]
            lrow = lin_pool.tile([1, QB], F32, name="lrow")
            nc.vector.tensor_copy(out=lrow[:], in_=ctp[DK : DK + 1, :])
            linv = lin_pool.tile([1, QB], F32, name="linv")
            # NB: approx_fast misreads PSUM operands on HW - keep in_ in SBUF
            nc.vector.reciprocal_approx_fast(out=linv[:], in_=lrow[:])
            linv16 = lin_pool.tile([1, QB], F16, name="linv16")
            nc.vector.tensor_copy(out=linv16[:], in_=linv[:])
            ctu = ctu_pool.tile([DK, QB], F32, name="ctu")
            nc.vector.tensor_copy(out=ctu[:], in_=ctp[0:DK, :])

            def norm():
                # broadcast 1/denom across partitions via K=1 f16 matmul
                # into the upper half of the ctp bank (one PSUM operand max
                # per DVE op, so ctx was copied to SBUF above)
                nc.tensor.matmul(
                    ctp[DK : DK + DK, :],
                    ones_sb[:],
                    linv16[:],
                    start=True,
                    stop=True,
                    tile_position=(0, 64),
                )
                nc.vector.tensor_tensor(
                    ctn_sb[hl * DK : (hl + 1) * DK, hp, :],
                    ctu[:],
                    ctp[DK : DK + DK, :],
                    ALU.mult,
                )
                if hp == 1 and hl == 1:  # block i fully normalized -> out-proj
                    for qs in range(4):
                        for nch in range(2):
                            fillers.append(
                                lambda i=i, c=ctn_sb, qs=qs, nch=nch: outproj_granule(
                                    i, c, qs, nch
                                )
                            )

            normq.append((uidx, norm))

        def emit_pv(u, pt, uidx):
            i, hp, g, hl = u
            njt = 4 * (i + 1)
            if g == 0 and hl == 0:
                ctp_cur[hp] = [
                    ct_ps.tile([128, QB], F32, name=f"ctp{t}") for t in range(2)
                ]
            ctp = ctp_cur[hp][hl]
            for jj in range(GK):
                j = g * GK + jj
                nc.tensor.matmul(
                    ctp[0 : DK + 1, :],
                    v_sb[j][:, hp * 2 + hl, :],
                    pt[:, jj, :],
                    start=(j == 0),
                    stop=(j == njt - 1),
                )
            if g == 2 * i + 1:  # this head's last PV -> normalize
                drain_norm(i, hp, hl, ctp, uidx)

        units = []
        for i in range(NQB):
            for hp in range(2):
                for g in range(2 * i + 2):
                    for hl in range(2):
                        units.append((i, hp, g, hl))

        cur_block = -1
        for uidx, u in enumerate(units):
            i, hp, g, hl = u
            if i != cur_block:
                cur_block = i
                # V-projection for the NEXT block's k-tiles rides as filler
                for j in range(4 * (i + 1), min(4 * (i + 2), NKT)):
                    fillers.append(lambda j=j: vproj_half(j, 0))
                    fillers.append(lambda j=j: vproj_half(j, 1))
            # ST first: the PE instruction in front is always the one the
            # ACT stream is waiting on, so exp runs gapless; PV/fillers
            # execute behind it during the exp itself.
            lo, hi = hl * 64, hl * 64 + 64
            stm = st_ps.tile([128, GK, QB], F32, name="stm")
            for jj in range(GK):
                j = g * GK + jj
                nc.tensor.matmul(
                    stm[:, jj, :],
                    kt_sb[lo:hi, hp, j * 128 : (j + 1) * 128],
                    qt_sb[lo:hi, hp, i * QB : (i + 1) * QB],
                    start=True,
                    stop=True,
                    tile_position=(lo, 0),
                )
            pt = pt_pool.tile([128, GK, QB], F16)
            trim = max(0, 128 * (g * GK - 4 * i))  # no q < trim is unmasked
            if trim > 0:
                # exp only the potentially-valid q range; affine_select
                # below zero-fills the whole tile wherever its predicate
                # is false, which covers the untouched q < trim region.
                nc.scalar.activation(
                    pt[:, :, trim:QB], stm[:, :, trim:QB], AF.Exp
                )
            else:
                nc.scalar.activation(pt[:], stm[:], AF.Exp)
            if g * GK >= 4 * i:
                # diagonal group: keep (p, jj, f) iff
                # f - p - 128*(g*GK - 4i) - 128*jj >= 0
                nc.gpsimd.affine_select(
                    pt[:],
                    pt[:],
                    pattern=[[-128, GK], [1, QB]],
                    compare_op=ALU.is_ge,
                    fill=0.0,
                    base=-(128 * (g * GK - 4 * i)),
                    channel_multiplier=-1,
                )
            pend.append((u, pt))
            if len(pend) > DEPTH:
                emit_pv(*pend.popleft(), uidx)
            if normq and normq[0][0] <= uidx - 2:
                normq.popleft()[1]()
            if fillers:
                fillers.popleft()()

        uidx = len(units)
        while pend:
            emit_pv(*pend.popleft(), uidx)
            uidx += 1
        while normq:
            normq.popleft()[1]()
        while fillers:  # out-proj of the last block
            fillers.popleft()()


_PROG = None


def _get_prog():
    global _PROG
    if _PROG is None:
        _PROG = build_program()
    return _PROG


def _wlayout(w):
    """[(n_out*128), f] -> [128, n_out, f] contiguous fp16 (device lhsT layout)."""
    n_out = w.shape[0] // 128
    return np.ascontiguousarray(
        w.reshape(n_out, 128, w.shape[1]).transpose(1, 0, 2)
    ).astype(np.float16)


def shard_inputs(X1, X2, WQ, WK, WV, WO):
    X1 = np.asarray(X1, dtype=np.float32)
    X2 = np.asarray(X2, dtype=np.float32)
    WQ = np.asarray(WQ, dtype=np.float32)
    WK = np.asarray(WK, dtype=np.float32)
    WV = np.asarray(WV, dtype=np.float32)
    WO = np.asarray(WO, dtype=np.float32)
    x1t = [np.ascontiguousarray(X1[b].T).astype(np.float16) for b in range(B)]
    x2t = [np.ascontiguousarray(X2[b].T).astype(np.float16) for b in range(B)]
    in_maps = []
    for c in range(NCORES):
        b, g = divmod(c, GROUPS)
        sl = slice(g * DG, (g + 1) * DG)
        in_maps.append(
            {
                "x1t": x1t[b],
                "x2t": x2t[b],
                # fold the 1/sqrt(DK) score scale into WQ (exact: power of 2)
                "wq": _wlayout(WQ[:, sl] * np.float32(0.125)),
                "wk": _wlayout(WK[:, sl]),
                "wv": _wlayout(WV[:, sl]),
                "wo": _wlayout(WO[sl, :]),
            }
        )
    return in_maps


LAST_RESULT = None


def kernel(X1, X2, padding_mask, WQ, WK, WV, WO, **kwargs):
    global LAST_RESULT
    del padding_mask  # all-False by construction (spec fill: zeros)
    nc = _get_prog()
    in_maps = shard_inputs(X1, X2, WQ, WK, WV, WO)
    res = run_bass_kernel_spmd(nc, in_maps, list(range(NCORES)), **kwargs)
    LAST_RESULT = res
    out = np.zeros((B, T, D), dtype=np.float32)
    for c in range(NCORES):
        out[c // GROUPS] += res.results[c]["out"]
    return out


# revision 31
# speedup vs baseline: 1.1609x; 1.0019x over previous
"""Trainium2 Bass kernel for multi-head attention (B=2, T=2048, D=1024, H=16).

Sharding (Megatron-style): 8 cores = 2 batches x 4 head-groups. Core c handles
batch c//4 and heads [4*(c%4), 4*(c%4)+4): WQ/WK/WV split column-wise (256
cols per core), WO split row-wise. Each core writes a (T, D) fp16 partial
output; the host sums the 4 partials per batch.

Device schedule (fp16 matmul operands, fp32 PSUM):
- A1/A2: Q^T/K^T projections in [d, q] layout (d on partitions), m-outer so
  the PE chases the X DMAs; PSUM->SBUF copies on DVE so ACT stays exp-only.
- Attention is one global conveyor over units (i, hp, g, hl):
  ST (PE, two heads packed into 64-row PE groups via tile_position) ->
  exp (ACT) -> causal mask (gpsimd affine_select, diagonal units only) ->
  PV (PE; V carries a ones column so PSUM row DK accumulates the softmax
  denominator). PV lags ST by DEPTH units, and V-projection/out-projection
  matmuls are interleaved as PE filler, so the ACT-bound softmax never
  leaves the PE idle (keeps the HAM clock warm).
- Per-head normalization starts the moment that head's PV finishes:
  denominator row -> reciprocal_approx_fast (DVE) -> partition_broadcast
  (gpsimd) -> multiply (DVE) -> out-proj granules enqueued as filler.
"""

from collections import deque

import numpy as np

import concourse.mybir as mybir
import concourse.tile as tile
from concourse import bacc
from concourse.bass_utils import run_bass_kernel_spmd
from concourse._compat import get_trn_type

F32 = mybir.dt.float32
F32R = mybir.dt.float32r
F16 = mybir.dt.float16
AF = mybir.ActivationFunctionType
ALU = mybir.AluOpType

B, T, D, H = 2, 2048, 1024, 16
DK = 64
NCORES = 8
GROUPS = 4          # head-groups = cores per batch
DG = 256            # d-columns per core (4 heads x 64)
NH = 4              # heads per core
QB = 512            # query block
NQB = T // QB       # 4
KTILE = 128
NKT = T // KTILE    # 16
NMT = D // 128      # 8 contraction tiles over D
GK = 2              # k-tiles per conveyor unit
DEPTH = 6           # PV lag in conveyor units


def build_program():
    nc = bacc.Bacc(get_trn_type() or "TRN2", target_bir_lowering=False, debug=False)

    x1t = nc.dram_tensor("x1t", [D, T], F16, kind="ExternalInput").ap()
    x2t = nc.dram_tensor("x2t", [D, T], F16, kind="ExternalInput").ap()
    wq = nc.dram_tensor("wq", [128, NMT, DG], F16, kind="ExternalInput").ap()
    wk = nc.dram_tensor("wk", [128, NMT, DG], F16, kind="ExternalInput").ap()
    wv = nc.dram_tensor("wv", [128, NMT, DG], F16, kind="ExternalInput").ap()
    wo = nc.dram_tensor("wo", [128, 2, D], F16, kind="ExternalInput").ap()
    out = nc.dram_tensor("out", [T, D], F16, kind="ExternalOutput").ap()

    with tile.TileContext(nc) as tc:
        _emit(nc, tc, x1t, x2t, wq, wk, wv, wo, out)
    nc.compile()
    return nc


def _emit(nc, tc, x1t, x2t, wq, wk, wv, wo, out):
    from contextlib import ExitStack

    with ExitStack() as ctx:
        wpool = ctx.enter_context(tc.tile_pool(name="weights", bufs=1))
        qkv = ctx.enter_context(tc.tile_pool(name="qkv", bufs=1))

        # --- weights to SBUF (gpsimd queue; wq first: it gates the first MMs)
        wq_sb = wpool.tile([128, NMT, DG], F16)
        nc.gpsimd.dma_start(wq_sb[:], wq[:])
        wk_sb = wpool.tile([128, NMT, DG], F16)
        nc.gpsimd.dma_start(wk_sb[:], wk[:])
        wv_sb = wpool.tile([128, NMT, DG], F16)
        nc.gpsimd.dma_start(wv_sb[:], wv[:])
        wo_sb = wpool.tile([128, 2, D], F16)
        nc.gpsimd.dma_start(wo_sb[:], wo[:])

        # Residents: QT/KT as [128, dt, q]; V as per-j tiles [k, h, DK+1]
        qt_sb = qkv.tile([128, 2, T], F16)
        kt_sb = qkv.tile([128, 2, T], F16)
        v_sb = [qkv.tile([128, NH, DK + 1], F16, name=f"vsb{j}") for j in range(NKT)]
        for j in range(NKT):
            nc.gpsimd.memset(v_sb[j][:, :, DK : DK + 1], 1.0)

        # X2 resident (V-projection granules run inside the conveyor).
        # x2 DMAs go on the sync queue AFTER x1's so the two streams don't
        # halve each other's HBM bandwidth (A1 is gated by x1 alone).
        x2_sb = qkv.tile([128, NMT, T], F16)

        # ones row for the PE-side denominator broadcast (K=1 f16 matmul)
        ones_sb = wpool.tile([1, DK], F16)
        nc.vector.memset(ones_sb[:], 1.0)

        # --- A1: QT = WQ^T X1^T, m-outer so matmuls chase the x1 DMAs ---
        with tc.tile_pool(name="x1pool", bufs=1) as x1pool, tc.tile_pool(
            name="psA", bufs=1, space="PSUM"
        ) as psA:
            qps = [psA.tile([128, NQB, QB], F32, name=f"qps{t}") for t in range(2)]
            x1_sb = x1pool.tile([128, NMT, T], F16)
            for m in range(NMT):
                nc.sync.dma_start(x1_sb[:, m, :], x1t[m * 128 : (m + 1) * 128, :])
            for m in range(NMT):  # x2 queued behind all of x1
                nc.sync.dma_start(x2_sb[:, m, :], x2t[m * 128 : (m + 1) * 128, :])
            # dt-outer: dt0's PSUM drains (ACT+DVE split) while dt1's
            # matmuls run, so the banks recycle into the next pool early
            for dt in range(2):
                for m in range(NMT):
                    lhsT = wq_sb[:, m, dt * 128 : (dt + 1) * 128]
                    for qc in range(NQB):
                        nc.tensor.matmul(
                            qps[dt][:, qc, :],
                            lhsT,
                            x1_sb[:, m, qc * QB : (qc + 1) * QB],
                            start=(m == 0),
                            stop=(m == NMT - 1),
                        )
                for qc in range(NQB):
                    if qc % 2 == 0:
                        nc.scalar.copy(
                            qt_sb[:, dt, qc * QB : (qc + 1) * QB],
                            qps[dt][:, qc, :],
                        )
                    else:
                        nc.vector.tensor_copy(
                            out=qt_sb[:, dt, qc * QB : (qc + 1) * QB],
                            in_=qps[dt][:, qc, :],
                        )

        # --- A2: KT (x2 resident by now) ---
        with tc.tile_pool(name="psK", bufs=1, space="PSUM") as psK:
            kps = [psK.tile([128, NQB, QB], F32, name=f"kps{t}") for t in range(2)]
            for dt in range(2):
                for m in range(NMT):
                    lhsT = wk_sb[:, m, dt * 128 : (dt + 1) * 128]
                    for kc in range(NQB):
                        nc.tensor.matmul(
                            kps[dt][:, kc, :],
                            lhsT,
                            x2_sb[:, m, kc * QB : (kc + 1) * QB],
                            start=(m == 0),
                            stop=(m == NMT - 1),
                        )
                for kc in range(NQB):
                    if kc % 2 == 0:
                        nc.scalar.copy(
                            kt_sb[:, dt, kc * QB : (kc + 1) * QB],
                            kps[dt][:, kc, :],
                        )
                    else:
                        nc.vector.tensor_copy(
                            out=kt_sb[:, dt, kc * QB : (kc + 1) * QB],
                            in_=kps[dt][:, kc, :],
                        )

        # --- attention-era pools (PSUM: 4 + 2 + 1 + 1 = 8 banks) ---
        st_ps = ctx.enter_context(tc.tile_pool(name="st_ps", bufs=2, space="PSUM"))
        ct_ps = ctx.enter_context(tc.tile_pool(name="ct_ps", bufs=1, space="PSUM"))
        op_ps = ctx.enter_context(tc.tile_pool(name="op_ps", bufs=1, space="PSUM"))
        vp_ps = ctx.enter_context(tc.tile_pool(name="vp_ps", bufs=1, space="PSUM"))
        pt_pool = ctx.enter_context(tc.tile_pool(name="pt", bufs=8))
        lin_pool = ctx.enter_context(tc.tile_pool(name="lin", bufs=4))
        ctu_pool = ctx.enter_context(tc.tile_pool(name="ctu", bufs=4))
        ctn_pool = ctx.enter_context(tc.tile_pool(name="ctn", bufs=2))
        osb_pool = ctx.enter_context(tc.tile_pool(name="osb", bufs=3))

        # ---- V-projection granules (two 4-MM halves per k-tile j) ----
        vp_state = {}

        def vproj_half(j, half):
            if half == 0:
                vp_state[j] = vp_ps.tile([128, QB], F32, name="vps")
            vps = vp_state[j]
            for m in range(half * 4, half * 4 + 4):
                nc.tensor.matmul(
                    vps[:, 0:DG],
                    x2_sb[:, m, j * 128 : (j + 1) * 128],
                    wv_sb[:, m, :],
                    start=(m == 0),
                    stop=(m == NMT - 1),
                )
            if half == 1:
                for h in range(NH):
                    nc.vector.tensor_copy(
                        out=v_sb[j][:, h, 0:DK], in_=vps[:, h * DK : (h + 1) * DK]
                    )
                del vp_state[j]

        for j in range(4):  # needed by block i=0, before the conveyor
            vproj_half(j, 0)
            vproj_half(j, 1)

        # ---- out-projection granules (2 accumulating MMs + copy + DMA);
        # granules alternate between the op and vproj PSUM banks so
        # consecutive granules double-buffer without an extra bank ----
        op_count = [0]

        def outproj_granule(i, ctn_sb, qs, nch):
            op_count[0] += 1
            if op_count[0] % 2 == 0:
                ops = op_ps.tile([128, QB], F32, name="ops")
            else:
                ops = vp_ps.tile([128, QB], F32, name="vps")
            for dt in range(2):
                nc.tensor.matmul(
                    ops[:],
                    ctn_sb[:, dt, qs * 128 : (qs + 1) * 128],
                    wo_sb[:, dt, nch * QB : (nch + 1) * QB],
                    start=(dt == 0),
                    stop=(dt == 1),
                )
            osb = osb_pool.tile([128, QB], F16, name="osb")
            nc.vector.tensor_copy(out=osb[:], in_=ops[:])
            nc.sync.dma_start(
                out[
                    i * QB + qs * 128 : i * QB + (qs + 1) * 128,
                    nch * QB : (nch + 1) * QB,
                ],
                osb[:],
            )

        # ---- conveyor state ----
        fillers = deque()
        pend = deque()
        normq = deque()  # (enqueue_unit_idx, closure) - run >=2 units later
        ctp_cur = {}   # hp -> [ctp_hl0, ctp_hl1] for the active (i, hp)
        ctn_cur = {}   # i -> ctn tile

        def drain_norm(i, hp, hl, ctp, uidx):
            """Head (2*hp+hl) of block i: reciprocal of the denominator row,
            then (deferred) PE-broadcast into rows 64:128 of the same PSUM
            bank and a fused PSUM-read multiply into ctn."""
            if i not in ctn_cur:
                ctn_cur[i] = ctn_pool.tile([128, 2, QB], F16, name="ctn")
            ctn_sb = ctn_cur[i]
            lrow = lin_pool.tile([1, QB], F32, name="lrow")
            linv = lin_pool.tile([1, QB], F32, name="linv")
            linv16 = lin_pool.tile([1, QB], F16, name="linv16")
            ctu = ctu_pool.tile([DK, QB], F32, name="ctu")
            if i == NQB - 1:
                # last block: ACT is idle by now - run the row copies there
                # so the tail's per-head chain isn't DVE-serial
                nc.scalar.copy(lrow[:], ctp[DK : DK + 1, :])
                # NB: approx_fast misreads PSUM on HW - in_ must be SBUF
                nc.vector.reciprocal_approx_fast(out=linv[:], in_=lrow[:])
                nc.scalar.copy(linv16[:], linv[:])
            else:
                nc.vector.tensor_copy(out=lrow[:], in_=ctp[DK : DK + 1, :])
                nc.vector.reciprocal_approx_fast(out=linv[:], in_=lrow[:])
                nc.vector.tensor_copy(out=linv16[:], in_=linv[:])
            nc.vector.tensor_copy(out=ctu[:], in_=ctp[0:DK, :])

            def norm():
                # broadcast 1/denom across partitions via K=1 f16 matmul
                # into the upper half of the ctp bank (one PSUM operand max
                # per DVE op, so ctx was copied to SBUF above)
                nc.tensor.matmul(
                    ctp[DK : DK + DK, :],
                    ones_sb[:],
                    linv16[:],
                    start=True,
                    stop=True,
                    tile_position=(0, 64),
                )
                nc.vector.tensor_tensor(
                    ctn_sb[hl * DK : (hl + 1) * DK, hp, :],
                    ctu[:],
                    ctp[DK : DK + DK, :],
                    ALU.mult,
                )
                if hp == 1 and hl == 1:  # block i fully normalized -> out-proj
                    for qs in range(4):
                        for nch in range(2):
                            fillers.append(
                                lambda i=i, c=ctn_sb, qs=qs, nch=nch: outproj_granule(
                                    i, c, qs, nch
                                )
                            )

            normq.append((uidx, norm))

        def emit_pv(u, pt, uidx):
            i, hp, g, hl = u
            njt = 4 * (i + 1)
            if g == 0 and hl == 0:
                ctp_cur[hp] = [
                    ct_ps.tile([128, QB], F32, name=f"ctp{t}") for t in range(2)
                ]
            ctp = ctp_cur[hp][hl]
            for jj in range(GK):
                j = g * GK + jj
                nc.tensor.matmul(
                    ctp[0 : DK + 1, :],
                    v_sb[j][:, hp * 2 + hl, :],
                    pt[:, jj, :],
                    start=(j == 0),
                    stop=(j == njt - 1),
                )
            if g == 2 * i + 1:  # this head's last PV -> normalize
                drain_norm(i, hp, hl, ctp, uidx)

        units = []
        for i in range(NQB):
            for hp in range(2):
                for g in range(2 * i + 2):
                    for hl in range(2):
                        units.append((i, hp, g, hl))

        cur_block = -1
        for uidx, u in enumerate(units):
            i, hp, g, hl = u
            if i != cur_block:
                cur_block = i
                # V-projection for the NEXT block's k-tiles rides as filler
                for j in range(4 * (i + 1), min(4 * (i + 2), NKT)):
                    fillers.append(lambda j=j: vproj_half(j, 0))
                    fillers.append(lambda j=j: vproj_half(j, 1))
            # ST first: the PE instruction in front is always the one the
            # ACT stream is waiting on, so exp runs gapless; PV/fillers
            # execute behind it during the exp itself.
            lo, hi = hl * 64, hl * 64 + 64
            stm = st_ps.tile([128, GK, QB], F32, name="stm")
            for jj in range(GK):
                j = g * GK + jj
                nc.tensor.matmul(
                    stm[:, jj, :],
                    kt_sb[lo:hi, hp, j * 128 : (j + 1) * 128],
                    qt_sb[lo:hi, hp, i * QB : (i + 1) * QB],
                    start=True,
                    stop=True,
                    tile_position=(lo, 0),
                )
            pt = pt_pool.tile([128, GK, QB], F16)
            trim = max(0, 128 * (g * GK - 4 * i))  # no q < trim is unmasked
            if trim > 0:
                # exp only the potentially-valid q range; affine_select
                # below zero-fills the whole tile wherever its predicate
                # is false, which covers the untouched q < trim region.
                nc.scalar.activation(
                    pt[:, :, trim:QB], stm[:, :, trim:QB], AF.Exp
                )
            else:
                nc.scalar.activation(pt[:], stm[:], AF.Exp)
            if g * GK >= 4 * i:
                # diagonal group: keep (p, jj, f) iff
                # f - p - 128*(g*GK - 4i) - 128*jj >= 0
                nc.gpsimd.affine_select(
                    pt[:],
                    pt[:],
                    pattern=[[-128, GK], [1, QB]],
                    compare_op=ALU.is_ge,
                    fill=0.0,
                    base=-(128 * (g * GK - 4 * i)),
                    channel_multiplier=-1,
                )
            pend.append((u, pt))
            if len(pend) > DEPTH:
                emit_pv(*pend.popleft(), uidx)
            if normq and normq[0][0] <= uidx - 2:
                normq.popleft()[1]()
            if fillers:
                fillers.popleft()()

        uidx = len(units)
        while pend:
            emit_pv(*pend.popleft(), uidx)
            uidx += 1
        while normq:
            normq.popleft()[1]()
        while fillers:  # out-proj of the last block
            fillers.popleft()()


_PROG = None


def _get_prog():
    global _PROG
    if _PROG is None:
        _PROG = build_program()
    return _PROG


def _wlayout(w):
    """[(n_out*128), f] -> [128, n_out, f] contiguous fp16 (device lhsT layout)."""
    n_out = w.shape[0] // 128
    return np.ascontiguousarray(
        w.reshape(n_out, 128, w.shape[1]).transpose(1, 0, 2)
    ).astype(np.float16)


def shard_inputs(X1, X2, WQ, WK, WV, WO):
    X1 = np.asarray(X1, dtype=np.float32)
    X2 = np.asarray(X2, dtype=np.float32)
    WQ = np.asarray(WQ, dtype=np.float32)
    WK = np.asarray(WK, dtype=np.float32)
    WV = np.asarray(WV, dtype=np.float32)
    WO = np.asarray(WO, dtype=np.float32)
    x1t = [np.ascontiguousarray(X1[b].T).astype(np.float16) for b in range(B)]
    x2t = [np.ascontiguousarray(X2[b].T).astype(np.float16) for b in range(B)]
    in_maps = []
    for c in range(NCORES):
        b, g = divmod(c, GROUPS)
        sl = slice(g * DG, (g + 1) * DG)
        in_maps.append(
            {
                "x1t": x1t[b],
                "x2t": x2t[b],
                # fold the 1/sqrt(DK) score scale into WQ (exact: power of 2)
                "wq": _wlayout(WQ[:, sl] * np.float32(0.125)),
                "wk": _wlayout(WK[:, sl]),
                "wv": _wlayout(WV[:, sl]),
                "wo": _wlayout(WO[sl, :]),
            }
        )
    return in_maps


LAST_RESULT = None


def kernel(X1, X2, padding_mask, WQ, WK, WV, WO, **kwargs):
    global LAST_RESULT
    del padding_mask  # all-False by construction (spec fill: zeros)
    nc = _get_prog()
    in_maps = shard_inputs(X1, X2, WQ, WK, WV, WO)
    res = run_bass_kernel_spmd(nc, in_maps, list(range(NCORES)), **kwargs)
    LAST_RESULT = res
    out = np.zeros((B, T, D), dtype=np.float32)
    for c in range(NCORES):
        out[c // GROUPS] += res.results[c]["out"]
    return out


# revision 32
# speedup vs baseline: 1.1991x; 1.0329x over previous
"""Trainium2 Bass kernel for multi-head attention (B=2, T=2048, D=1024, H=16).

Sharding (Megatron-style): 8 cores = 2 batches x 4 head-groups. Core c handles
batch c//4 and heads [4*(c%4), 4*(c%4)+4): WQ/WK/WV split column-wise (256
cols per core), WO split row-wise. Each core writes a (T, D) fp16 partial
output; the host sums the 4 partials per batch.

Device schedule (fp16 matmul operands, fp32 PSUM):
- A1/A2: Q^T/K^T projections in [d, q] layout (d on partitions), m-outer so
  the PE chases the X DMAs; PSUM->SBUF copies on DVE so ACT stays exp-only.
- Attention is one global conveyor over units (i, hp, g, hl):
  ST (PE, two heads packed into 64-row PE groups via tile_position) ->
  exp (ACT) -> causal mask (gpsimd affine_select, diagonal units only) ->
  PV (PE; V carries a ones column so PSUM row DK accumulates the softmax
  denominator). PV lags ST by DEPTH units, and V-projection/out-projection
  matmuls are interleaved as PE filler, so the ACT-bound softmax never
  leaves the PE idle (keeps the HAM clock warm).
- Per-head normalization starts the moment that head's PV finishes:
  denominator row -> reciprocal_approx_fast (DVE) -> partition_broadcast
  (gpsimd) -> multiply (DVE) -> out-proj granules enqueued as filler.
"""

from collections import deque

import numpy as np

import concourse.mybir as mybir
import concourse.tile as tile
from concourse import bacc
from concourse.bass_utils import run_bass_kernel_spmd
from concourse._compat import get_trn_type

F32 = mybir.dt.float32
F32R = mybir.dt.float32r
F16 = mybir.dt.float16
AF = mybir.ActivationFunctionType
ALU = mybir.AluOpType

B, T, D, H = 2, 2048, 1024, 16
DK = 64
NCORES = 8
GROUPS = 4          # head-groups = cores per batch
DG = 256            # d-columns per core (4 heads x 64)
NH = 4              # heads per core
QB = 512            # query block
NQB = T // QB       # 4
KTILE = 128
NKT = T // KTILE    # 16
NMT = D // 128      # 8 contraction tiles over D
GK = 2              # k-tiles per conveyor unit
DEPTH = 6           # PV lag in conveyor units


def build_program():
    nc = bacc.Bacc(get_trn_type() or "TRN2", target_bir_lowering=False, debug=False)

    x1t = nc.dram_tensor("x1t", [D, T], F16, kind="ExternalInput").ap()
    x2t = nc.dram_tensor("x2t", [D, T], F16, kind="ExternalInput").ap()
    wq = nc.dram_tensor("wq", [128, NMT, DG], F16, kind="ExternalInput").ap()
    wk = nc.dram_tensor("wk", [128, NMT, DG], F16, kind="ExternalInput").ap()
    wv = nc.dram_tensor("wv", [128, NMT, DG], F16, kind="ExternalInput").ap()
    wo = nc.dram_tensor("wo", [128, 2, D], F16, kind="ExternalInput").ap()
    out = nc.dram_tensor("out", [T, D], F16, kind="ExternalOutput").ap()

    with tile.TileContext(nc) as tc:
        _emit(nc, tc, x1t, x2t, wq, wk, wv, wo, out)
    nc.compile()
    return nc


def _emit(nc, tc, x1t, x2t, wq, wk, wv, wo, out):
    from contextlib import ExitStack

    with ExitStack() as ctx:
        wpool = ctx.enter_context(tc.tile_pool(name="weights", bufs=1))
        qkv = ctx.enter_context(tc.tile_pool(name="qkv", bufs=1))

        # --- weights to SBUF (gpsimd queue; wq first: it gates the first MMs)
        wq_sb = wpool.tile([128, NMT, DG], F16)
        nc.gpsimd.dma_start(wq_sb[:], wq[:])
        wk_sb = wpool.tile([128, NMT, DG], F16)
        nc.gpsimd.dma_start(wk_sb[:], wk[:])
        wv_sb = wpool.tile([128, NMT, DG], F16)
        nc.gpsimd.dma_start(wv_sb[:], wv[:])
        wo_sb = wpool.tile([128, 2, D], F16)
        nc.gpsimd.dma_start(wo_sb[:], wo[:])

        # Residents: QT/KT as [128, dt, q]; V as per-j tiles [k, h, DK+1]
        qt_sb = qkv.tile([128, 2, T], F16)
        kt_sb = qkv.tile([128, 2, T], F16)
        v_sb = [qkv.tile([128, NH, DK + 1], F16, name=f"vsb{j}") for j in range(NKT)]
        for j in range(NKT):
            nc.gpsimd.memset(v_sb[j][:, :, DK : DK + 1], 1.0)

        # X2 resident (V-projection granules run inside the conveyor).
        # x2 DMAs go on the sync queue AFTER x1's so the two streams don't
        # halve each other's HBM bandwidth (A1 is gated by x1 alone).
        x2_sb = qkv.tile([128, NMT, T], F16)

        # ones row for the PE-side denominator broadcast (K=1 f16 matmul)
        ones_sb = wpool.tile([1, DK], F16)
        nc.vector.memset(ones_sb[:], 1.0)

        # --- A1: QT = WQ^T X1^T, m-outer so matmuls chase the x1 DMAs ---
        with tc.tile_pool(name="x1pool", bufs=1) as x1pool, tc.tile_pool(
            name="psA", bufs=1, space="PSUM"
        ) as psA:
            qps = [psA.tile([128, NQB, QB], F32, name=f"qps{t}") for t in range(2)]
            x1_sb = x1pool.tile([128, NMT, T], F16)
            for m in range(NMT):
                nc.sync.dma_start(x1_sb[:, m, :], x1t[m * 128 : (m + 1) * 128, :])
            for m in range(NMT):  # x2 queued behind all of x1
                nc.sync.dma_start(x2_sb[:, m, :], x2t[m * 128 : (m + 1) * 128, :])
            # dt-outer: dt0's PSUM drains (ACT+DVE split) while dt1's
            # matmuls run, so the banks recycle into the next pool early
            for dt in range(2):
                for m in range(NMT):
                    lhsT = wq_sb[:, m, dt * 128 : (dt + 1) * 128]
                    for qc in range(NQB):
                        nc.tensor.matmul(
                            qps[dt][:, qc, :],
                            lhsT,
                            x1_sb[:, m, qc * QB : (qc + 1) * QB],
                            start=(m == 0),
                            stop=(m == NMT - 1),
                        )
                for qc in range(NQB):
                    if qc % 2 == 0:
                        nc.scalar.copy(
                            qt_sb[:, dt, qc * QB : (qc + 1) * QB],
                            qps[dt][:, qc, :],
                        )
                    else:
                        nc.vector.tensor_copy(
                            out=qt_sb[:, dt, qc * QB : (qc + 1) * QB],
                            in_=qps[dt][:, qc, :],
                        )

            # --- A2: KT reuses the SAME PSUM tiles (kps[dt] := qps[dt]) so
            # the K matmuls only WAR on that dt's qt copies, which finished
            # during the other dt's pass - no pool-boundary bank stall ---
            for dt in range(2):
                for m in range(NMT):
                    lhsT = wk_sb[:, m, dt * 128 : (dt + 1) * 128]
                    for kc in range(NQB):
                        nc.tensor.matmul(
                            qps[dt][:, kc, :],
                            lhsT,
                            x2_sb[:, m, kc * QB : (kc + 1) * QB],
                            start=(m == 0),
                            stop=(m == NMT - 1),
                        )
                for kc in range(NQB):
                    if kc % 2 == 0:
                        nc.scalar.copy(
                            kt_sb[:, dt, kc * QB : (kc + 1) * QB],
                            qps[dt][:, kc, :],
                        )
                    else:
                        nc.vector.tensor_copy(
                            out=kt_sb[:, dt, kc * QB : (kc + 1) * QB],
                            in_=qps[dt][:, kc, :],
                        )

        # --- attention-era pools (PSUM: 4 + 2 + 1 + 1 = 8 banks) ---
        st_ps = ctx.enter_context(tc.tile_pool(name="st_ps", bufs=2, space="PSUM"))
        ct_ps = ctx.enter_context(tc.tile_pool(name="ct_ps", bufs=1, space="PSUM"))
        op_ps = ctx.enter_context(tc.tile_pool(name="op_ps", bufs=1, space="PSUM"))
        vp_ps = ctx.enter_context(tc.tile_pool(name="vp_ps", bufs=1, space="PSUM"))
        pt_pool = ctx.enter_context(tc.tile_pool(name="pt", bufs=8))
        lin_pool = ctx.enter_context(tc.tile_pool(name="lin", bufs=4))
        ctu_pool = ctx.enter_context(tc.tile_pool(name="ctu", bufs=4))
        ctn_pool = ctx.enter_context(tc.tile_pool(name="ctn", bufs=2))
        osb_pool = ctx.enter_context(tc.tile_pool(name="osb", bufs=3))

        # ---- V-projection granules (two 4-MM halves per k-tile j) ----
        vp_state = {}

        def vproj_half(j, half):
            if half == 0:
                vp_state[j] = vp_ps.tile([128, QB], F32, name="vps")
            vps = vp_state[j]
            for m in range(half * 4, half * 4 + 4):
                nc.tensor.matmul(
                    vps[:, 0:DG],
                    x2_sb[:, m, j * 128 : (j + 1) * 128],
                    wv_sb[:, m, :],
                    start=(m == 0),
                    stop=(m == NMT - 1),
                )
            if half == 1:
                for h in range(NH):
                    nc.vector.tensor_copy(
                        out=v_sb[j][:, h, 0:DK], in_=vps[:, h * DK : (h + 1) * DK]
                    )
                del vp_state[j]

        for j in range(4):  # needed by block i=0, before the conveyor
            vproj_half(j, 0)
            vproj_half(j, 1)

        # ---- out-projection granules (2 accumulating MMs + copy + DMA);
        # granules alternate between the op and vproj PSUM banks so
        # consecutive granules double-buffer without an extra bank ----
        op_count = [0]

        def outproj_granule(i, ctn_sb, qs, nch):
            op_count[0] += 1
            if op_count[0] % 2 == 0:
                ops = op_ps.tile([128, QB], F32, name="ops")
            else:
                ops = vp_ps.tile([128, QB], F32, name="vps")
            for dt in range(2):
                nc.tensor.matmul(
                    ops[:],
                    ctn_sb[:, dt, qs * 128 : (qs + 1) * 128],
                    wo_sb[:, dt, nch * QB : (nch + 1) * QB],
                    start=(dt == 0),
                    stop=(dt == 1),
                )
            osb = osb_pool.tile([128, QB], F16, name="osb")
            nc.vector.tensor_copy(out=osb[:], in_=ops[:])
            nc.sync.dma_start(
                out[
                    i * QB + qs * 128 : i * QB + (qs + 1) * 128,
                    nch * QB : (nch + 1) * QB,
                ],
                osb[:],
            )

        # ---- conveyor state ----
        fillers = deque()
        pend = deque()
        normq = deque()  # (enqueue_unit_idx, closure) - run >=2 units later
        ctp_cur = {}   # hp -> [ctp_hl0, ctp_hl1] for the active (i, hp)
        ctn_cur = {}   # i -> ctn tile

        def drain_norm(i, hp, hl, ctp, uidx):
            """Head (2*hp+hl) of block i: reciprocal of the denominator row,
            then (deferred) PE-broadcast into rows 64:128 of the same PSUM
            bank and a fused PSUM-read multiply into ctn."""
            if i not in ctn_cur:
                ctn_cur[i] = ctn_pool.tile([128, 2, QB], F16, name="ctn")
            ctn_sb = ctn_cur[i]
            lrow = lin_pool.tile([1, QB], F32, name="lrow")
            linv = lin_pool.tile([1, QB], F32, name="linv")
            linv16 = lin_pool.tile([1, QB], F16, name="linv16")
            ctu = ctu_pool.tile([DK, QB], F32, name="ctu")
            if i == NQB - 1:
                # last block: ACT is idle by now - run the row copies there
                # so the tail's per-head chain isn't DVE-serial
                nc.scalar.copy(lrow[:], ctp[DK : DK + 1, :])
                # NB: approx_fast misreads PSUM on HW - in_ must be SBUF
                nc.vector.reciprocal_approx_fast(out=linv[:], in_=lrow[:])
                nc.scalar.copy(linv16[:], linv[:])
            else:
                nc.vector.tensor_copy(out=lrow[:], in_=ctp[DK : DK + 1, :])
                nc.vector.reciprocal_approx_fast(out=linv[:], in_=lrow[:])
                nc.vector.tensor_copy(out=linv16[:], in_=linv[:])
            nc.vector.tensor_copy(out=ctu[:], in_=ctp[0:DK, :])

            def norm():
                # broadcast 1/denom across partitions via K=1 f16 matmul
                # into the upper half of the ctp bank (one PSUM operand max
                # per DVE op, so ctx was copied to SBUF above)
                nc.tensor.matmul(
                    ctp[DK : DK + DK, :],
                    ones_sb[:],
                    linv16[:],
                    start=True,
                    stop=True,
                    tile_position=(0, 64),
                )
                nc.vector.tensor_tensor(
                    ctn_sb[hl * DK : (hl + 1) * DK, hp, :],
                    ctu[:],
                    ctp[DK : DK + DK, :],
                    ALU.mult,
                )
                if hp == 1 and hl == 1:  # block i fully normalized -> out-proj
                    for qs in range(4):
                        for nch in range(2):
                            fillers.append(
                                lambda i=i, c=ctn_sb, qs=qs, nch=nch: outproj_granule(
                                    i, c, qs, nch
                                )
                            )

            normq.append((uidx, norm))

        def emit_pv(u, pt, uidx):
            i, hp, g, hl = u
            njt = 4 * (i + 1)
            if g == 0 and hl == 0:
                ctp_cur[hp] = [
                    ct_ps.tile([128, QB], F32, name=f"ctp{t}") for t in range(2)
                ]
            ctp = ctp_cur[hp][hl]
            for jj in range(GK):
                j = g * GK + jj
                nc.tensor.matmul(
                    ctp[0 : DK + 1, :],
                    v_sb[j][:, hp * 2 + hl, :],
                    pt[:, jj, :],
                    start=(j == 0),
                    stop=(j == njt - 1),
                )
            if g == 2 * i + 1:  # this head's last PV -> normalize
                drain_norm(i, hp, hl, ctp, uidx)

        units = []
        for i in range(NQB):
            for hp in range(2):
                for g in range(2 * i + 2):
                    for hl in range(2):
                        units.append((i, hp, g, hl))

        cur_block = -1
        for uidx, u in enumerate(units):
            i, hp, g, hl = u
            if i != cur_block:
                cur_block = i
                # V-projection for the NEXT block's k-tiles rides as filler
                for j in range(4 * (i + 1), min(4 * (i + 2), NKT)):
                    fillers.append(lambda j=j: vproj_half(j, 0))
                    fillers.append(lambda j=j: vproj_half(j, 1))
            # ST first: the PE instruction in front is always the one the
            # ACT stream is waiting on, so exp runs gapless; PV/fillers
            # execute behind it during the exp itself.
            lo, hi = hl * 64, hl * 64 + 64
            stm = st_ps.tile([128, GK, QB], F32, name="stm")
            for jj in range(GK):
                j = g * GK + jj
                nc.tensor.matmul(
                    stm[:, jj, :],
                    kt_sb[lo:hi, hp, j * 128 : (j + 1) * 128],
                    qt_sb[lo:hi, hp, i * QB : (i + 1) * QB],
                    start=True,
                    stop=True,
                    tile_position=(lo, 0),
                )
            pt = pt_pool.tile([128, GK, QB], F16)
            trim = max(0, 128 * (g * GK - 4 * i))  # no q < trim is unmasked
            if trim > 0:
                # exp only the potentially-valid q range; affine_select
                # below zero-fills the whole tile wherever its predicate
                # is false, which covers the untouched q < trim region.
                nc.scalar.activation(
                    pt[:, :, trim:QB], stm[:, :, trim:QB], AF.Exp
                )
            else:
                nc.scalar.activation(pt[:], stm[:], AF.Exp)
            if g * GK >= 4 * i:
                # diagonal group: keep (p, jj, f) iff
                # f - p - 128*(g*GK - 4i) - 128*jj >= 0
                nc.gpsimd.affine_select(
                    pt[:],
                    pt[:],
                    pattern=[[-128, GK], [1, QB]],
                    compare_op=ALU.is_ge,
                    fill=0.0,
                    base=-(128 * (g * GK - 4 * i)),
                    channel_multiplier=-1,
                )
            pend.append((u, pt))
            if len(pend) > DEPTH:
                emit_pv(*pend.popleft(), uidx)
            if normq and normq[0][0] <= uidx - 2:
                normq.popleft()[1]()
            if fillers:
                fillers.popleft()()

        uidx = len(units)
        while pend:
            emit_pv(*pend.popleft(), uidx)
            uidx += 1
        while normq:
            normq.popleft()[1]()
        while fillers:  # out-proj of the last block
            fillers.popleft()()


_PROG = None


def _get_prog():
    global _PROG
    if _PROG is None:
        _PROG = build_program()
    return _PROG


def _wlayout(w):
    """[(n_out*128), f] -> [128, n_out, f] contiguous fp16 (device lhsT layout)."""
    n_out = w.shape[0] // 128
    return np.ascontiguousarray(
        w.reshape(n_out, 128, w.shape[1]).transpose(1, 0, 2)
    ).astype(np.float16)


def shard_inputs(X1, X2, WQ, WK, WV, WO):
    X1 = np.asarray(X1, dtype=np.float32)
    X2 = np.asarray(X2, dtype=np.float32)
    WQ = np.asarray(WQ, dtype=np.float32)
    WK = np.asarray(WK, dtype=np.float32)
    WV = np.asarray(WV, dtype=np.float32)
    WO = np.asarray(WO, dtype=np.float32)
    x1t = [np.ascontiguousarray(X1[b].T).astype(np.float16) for b in range(B)]
    x2t = [np.ascontiguousarray(X2[b].T).astype(np.float16) for b in range(B)]
    in_maps = []
    for c in range(NCORES):
        b, g = divmod(c, GROUPS)
        sl = slice(g * DG, (g + 1) * DG)
        in_maps.append(
            {
                "x1t": x1t[b],
                "x2t": x2t[b],
                # fold the 1/sqrt(DK) score scale into WQ (exact: power of 2)
                "wq": _wlayout(WQ[:, sl] * np.float32(0.125)),
                "wk": _wlayout(WK[:, sl]),
                "wv": _wlayout(WV[:, sl]),
                "wo": _wlayout(WO[sl, :]),
            }
        )
    return in_maps


LAST_RESULT = None


def kernel(X1, X2, padding_mask, WQ, WK, WV, WO, **kwargs):
    global LAST_RESULT
    del padding_mask  # all-False by construction (spec fill: zeros)
    nc = _get_prog()
    in_maps = shard_inputs(X1, X2, WQ, WK, WV, WO)
    res = run_bass_kernel_spmd(nc, in_maps, list(range(NCORES)), **kwargs)
    LAST_RESULT = res
    out = np.zeros((B, T, D), dtype=np.float32)
    for c in range(NCORES):
        out[c // GROUPS] += res.results[c]["out"]
    return out
